# revision 1
# baseline (speedup 1.0000x reference)
"""Trainium2 Bass kernel for nn_DGLGraphConv (graph conv with sum- and product-reduce).

Strategy (8 NeuronCores, SPMD, two launches):
  Launch A (node-sharded, 6250 nodes/core): per-node table
      T[n] = [ (feat@w1)*s_out | v ],  v = |log|tanh(((feat@w2')*s_out)+b)|| carrying tanh's sign bit
  Host: concat T shards (pure relayout; row 0 / tail rows are zeros used as gather padding).
  Launch B (dst-sharded by edge partitioning): per core, 49 windows x 128 dst slots.
      Edges sorted by (dst window, src half, src), padded to 128-edge blocks with
      block counts maxed over cores so all cores run one program. Per block:
      dma_gather 128 rows of T (768B each), one-hot S matmuls accumulate
      [sum(feat_sum) | sum|v| | count(v<0)] per dst slot in PSUM; epilogue computes
      sign via parity, exp, mask, @v, and the in-degree scaling.

Host does integer index prep (bincount/sort/pad) and layout-only transforms;
all floating-point math runs on device.
"""
import sys
from contextlib import ExitStack

import numpy as np

for _p in ("/opt/trn_rl_repo",):
    if _p not in sys.path:
        sys.path.insert(0, _p)

import concourse.bass as bass
import concourse.mybir as mybir
import concourse.tile as tile
from concourse import bacc, bass_utils
from concourse._compat import with_exitstack
from concourse.masks import make_identity

F32 = mybir.dt.float32
I32 = mybir.dt.int32
I16 = mybir.dt.int16
AF = mybir.ActivationFunctionType
ALU = mybir.AluOpType

MASK_ABS = 0x7FFFFFFF
MASK_SGN = -0x80000000
BF16 = mybir.dt.bfloat16
SPLIT_BF16 = True  # store T as [hi|lo] bf16 pairs; phase-B matmuls run in bf16
# T row layout (bf16, 512 cols = 1024B):
#   [fs_hi(128) | absv_hi(64) | neg(64) | fs_lo(128) | absv_lo(64) | pad(64)]
# mm1 rhs = cols 0:256 -> psum 0:256 (fs, absv, neg); mm2 rhs = cols 256:448
# -> psum 0:192 accumulate (fs_lo, absv_lo). One accumulation group per window.
TROW = 512
import os as _os
# Max 128-edge blocks per dma_gather call. Full-window calls (~2200 indices)
# die on HW (descriptor-ring overflow); 8 blocks = 1024 descriptors is safe.
GATHER_CHUNK = int(_os.environ.get("GCH", "8"))


# ---------------- host-side prep (integer/layout only) ----------------

def make_dims(N=50000, E=800000, DIN=256, DOUT=128, RANK=64, M=8, LO_MAX=32766):
    LO_MAX = min(LO_MAX, N - 1)
    NSH = N // M
    W = 128
    NW = (NSH + W - 1) // W
    T_ROWS = N + 4
    HI_BASE = LO_MAX + 2
    NHI = T_ROWS - HI_BASE
    return dict(N=N, E=E, DIN=DIN, DOUT=DOUT, RANK=RANK, M=M, NSH=NSH, W=W, NW=NW,
                LO_MAX=LO_MAX, T_ROWS=T_ROWS, HI_BASE=HI_BASE, NHI=NHI,
                HI_PAD_IDX=NHI - 2)


def preprocess(src, dst, dm):
    N, E, M, NSH, W, NW = dm["N"], dm["E"], dm["M"], dm["NSH"], dm["W"], dm["NW"]
    LO_MAX, HI_PAD_IDX = dm["LO_MAX"], dm["HI_PAD_IDX"]
    src = np.asarray(src).astype(np.int64)
    dst = np.asarray(dst).astype(np.int64)
    deg_out = np.bincount(src, minlength=N).astype(np.float32)
    deg_in = np.bincount(dst, minlength=N).astype(np.float32)

    order = np.lexsort((src, dst))
    s_srt, d_srt = src[order], dst[order]
    core_of = d_srt // NSH
    win_of = (d_srt - core_of * NSH) // W

    is_hi = s_srt > LO_MAX
    nlo = np.zeros((M, NW), np.int64)
    nhi = np.zeros((M, NW), np.int64)
    np.add.at(nlo, (core_of[~is_hi], win_of[~is_hi]), 1)
    np.add.at(nhi, (core_of[is_hi], win_of[is_hi]), 1)

    BL = np.maximum(1, -(-nlo.max(axis=0) // 128))
    BH = -(-nhi.max(axis=0) // 128)
    BT = BL + BH
    NB = int(BT.sum())

    idx16 = np.zeros((M, NB * 128), np.int16)
    dstloc = np.zeros((M, 128, NB), np.float32)
    key_half = is_hi.astype(np.int64)
    order2 = np.lexsort((s_srt, key_half, win_of, core_of))
    s2, d2 = s_srt[order2], d_srt[order2]
    c2, w2_, h2 = core_of[order2], win_of[order2], key_half[order2]

    woff = np.concatenate([[0], np.cumsum(BT)])
    counts = np.zeros((M, NW, 2), np.int64)
    np.add.at(counts, (c2, w2_, h2), 1)
    ptr = 0
    for k in range(M):
        for w in range(NW):
            cl, ch = counts[k, w, 0], counts[k, w, 1]
            base_blk = woff[w]
            seg = slice(ptr, ptr + cl)
            pos = np.arange(cl)
            idx16[k, base_blk * 128 + pos] = (s2[seg] + 1).astype(np.int16)
            dstloc[k, pos % 128, base_blk + pos // 128] = (d2[seg] - k * NSH - w * W).astype(np.float32)
            ptr += cl
            base_blk_h = woff[w] + BL[w]
            seg = slice(ptr, ptr + ch)
            pos = np.arange(ch)
            idx16[k, base_blk_h * 128 + pos] = (s2[seg] - LO_MAX - 1).astype(np.int16)
            dstloc[k, pos % 128, base_blk_h + pos // 128] = (d2[seg] - k * NSH - w * W).astype(np.float32)
            ptr += ch
            if ch < BH[w] * 128:
                pad = np.arange(ch, BH[w] * 128)
                idx16[k, base_blk_h * 128 + pad] = HI_PAD_IDX
    assert ptr == E

    idx16_wrapped = np.tile(idx16.reshape(M, -1, 16).transpose(0, 2, 1), (1, 8, 1)).copy()

    def shard_deg(deg):
        out = np.zeros((M, 128, NW), np.float32)
        for k in range(M):
            d = deg[k * NSH:(k + 1) * NSH]
            d = np.concatenate([d, np.zeros(NW * W - NSH, np.float32)])
            out[k] = d.reshape(NW, W).T
        return out

    return dict(BL=BL, BH=BH, BT=BT, NB=NB, idx16_wrapped=idx16_wrapped, dstloc=dstloc,
                deg_in_sh=shard_deg(deg_in), deg_out_sh=shard_deg(deg_out), woff=woff)


def host_phase_a_inputs(feat, w1, w2, pp, dm):
    M, NSH, DIN, DOUT, RANK = dm["M"], dm["NSH"], dm["DIN"], dm["DOUT"], dm["RANK"]
    featT = np.ascontiguousarray(np.asarray(feat).T)
    Wcat = np.ascontiguousarray(np.concatenate([w1, w2[:DIN]], axis=1), dtype=np.float32)
    brow = np.zeros((1, DOUT + RANK), np.float32)
    brow[0, DOUT:] = w2[DIN]
    return [dict(featT=np.ascontiguousarray(featT[:, k * NSH:(k + 1) * NSH]),
                 Wcat=Wcat, brow=brow, deg_out_sh=pp["deg_out_sh"][k])
            for k in range(M)]


def assemble_T(shards, dm):
    width = shards[0].shape[1]
    T = np.zeros((dm["T_ROWS"], width), shards[0].dtype)
    for k in range(dm["M"]):
        T[1 + k * dm["NSH"]:1 + (k + 1) * dm["NSH"]] = shards[k][:dm["NSH"]]
    return T


def host_phase_b_inputs(T, pp, vmat, dm):
    return [dict(T=T, idx16w=pp["idx16_wrapped"][k], dstloc=pp["dstloc"][k],
                 deg_in_sh=pp["deg_in_sh"][k], vmat=np.ascontiguousarray(vmat, np.float32))
            for k in range(dm["M"])]


# ---------------- device kernels ----------------

@with_exitstack
def build_phase_a(ctx, tc, outs, ins, cfg):
    nc = tc.nc
    NSH, NW = cfg["NSH"], cfg["NW"]
    DIN = cfg["DIN"]
    DO, RK = cfg["DOUT"], cfg["RANK"]
    DT = DO + RK
    KC = DIN // 128

    cpool = ctx.enter_context(tc.tile_pool(name="const", bufs=1))
    wpool = ctx.enter_context(tc.tile_pool(name="work", bufs=4))
    ppool = ctx.enter_context(tc.tile_pool(name="psum", bufs=4, space="PSUM"))

    featT = cpool.tile([128, KC, NSH], F32)
    nc.sync.dma_start(out=featT[:],
                      in_=ins["featT"].rearrange("(c p) n -> p c n", p=128))
    Wsb = cpool.tile([128, KC, DT], F32)
    nc.sync.dma_start(out=Wsb[:],
                      in_=ins["Wcat"].rearrange("(c p) n -> p c n", p=128))
    bsb = cpool.tile([1, DT], F32)
    nc.sync.dma_start(out=bsb[:], in_=ins["brow"][:])
    ones = cpool.tile([1, 128], F32)
    nc.vector.memset(ones[:], 1.0)
    # replicate bias across partitions once: brep = ones.T @ bsb
    bps = ppool.tile([128, DT], F32, tag="bps")
    nc.tensor.matmul(out=bps[:], lhsT=ones[:], rhs=bsb[:], start=True, stop=True)
    brep = cpool.tile([128, DT], F32)
    nc.vector.tensor_copy(out=brep[:], in_=bps[:])
    deg = cpool.tile([128, NW], F32)
    nc.sync.dma_start(out=deg[:], in_=ins["deg_out_sh"][:])
    s_all = cpool.tile([128, NW], F32)
    nc.vector.tensor_scalar(out=s_all[:], in0=deg[:], scalar1=1.0, scalar2=None, op0=ALU.max)
    nc.scalar.activation(out=s_all[:], in_=s_all[:], func=AF.Sqrt)
    nc.vector.reciprocal(out=s_all[:], in_=s_all[:])

    for t in range(NW):
        nt = min(128, NSH - t * 128)
        ps = ppool.tile([128, DT], F32, tag="ps")
        for c in range(KC):
            nc.tensor.matmul(out=ps[:nt, :], lhsT=featT[:, c, t * 128:t * 128 + nt],
                             rhs=Wsb[:, c, :], start=(c == 0), stop=(c == KC - 1))
        st = wpool.tile([128, DT], F32, tag="st")
        nc.vector.tensor_scalar(out=st[:nt, :], in0=ps[:nt, :], scalar1=s_all[:nt, t:t + 1],
                                scalar2=None, op0=ALU.mult)
        nc.vector.tensor_tensor(out=st[:nt, :], in0=st[:nt, :], in1=brep[:nt, :], op=ALU.add)
        tnh = wpool.tile([128, RK], F32, tag="tnh")
        nc.scalar.activation(out=tnh[:nt, :], in_=st[:nt, DO:DT], func=AF.Tanh)
        ab = wpool.tile([128, RK], F32, tag="ab")
        nc.vector.tensor_scalar(out=ab[:nt, :].bitcast(I32), in0=tnh[:nt, :].bitcast(I32),
                                scalar1=MASK_ABS, scalar2=None, op0=ALU.bitwise_and)
        ln = wpool.tile([128, RK], F32, tag="ln")
        nc.scalar.activation(out=ln[:nt, :], in_=ab[:nt, :], func=AF.Ln)
        nc.vector.tensor_scalar(out=ln[:nt, :], in0=ln[:nt, :], scalar1=-1e-7, scalar2=None, op0=ALU.min)
        nc.vector.tensor_scalar(out=ln[:nt, :].bitcast(I32), in0=ln[:nt, :].bitcast(I32),
                                scalar1=MASK_ABS, scalar2=None, op0=ALU.bitwise_and)
        sb = wpool.tile([128, RK], F32, tag="sb")
        nc.vector.tensor_scalar(out=sb[:nt, :].bitcast(I32), in0=tnh[:nt, :].bitcast(I32),
                                scalar1=MASK_SGN, scalar2=None, op0=ALU.bitwise_and)
        nc.vector.tensor_tensor(out=st[:nt, DO:DT].bitcast(I32), in0=ln[:nt, :].bitcast(I32),
                                in1=sb[:nt, :].bitcast(I32), op=ALU.bitwise_or)
        if not SPLIT_BF16:
            nc.sync.dma_start(out=outs["Tsh"][t * 128:t * 128 + nt, :], in_=st[:nt, :])
        else:
            sto = wpool.tile([128, TROW], BF16, tag="sto")
            # neg indicator from sign-encoded v, then make st = [fs | absv]
            nc.vector.tensor_scalar(out=sto[:nt, DT:256], in0=st[:nt, DO:DT],
                                    scalar1=0.0, scalar2=None, op0=ALU.is_lt)
            nc.vector.tensor_scalar(out=st[:nt, DO:DT].bitcast(I32),
                                    in0=st[:nt, DO:DT].bitcast(I32),
                                    scalar1=MASK_ABS, scalar2=None, op0=ALU.bitwise_and)
            nc.vector.tensor_copy(out=sto[:nt, 0:DT], in_=st[:nt, :])
            hi32 = wpool.tile([128, DT], F32, tag="hi32")
            nc.vector.tensor_copy(out=hi32[:nt, :], in_=sto[:nt, 0:DT])
            nc.vector.tensor_tensor(out=sto[:nt, 256:256 + DT], in0=st[:nt, :],
                                    in1=hi32[:nt, :], op=ALU.subtract)
            nc.vector.memset(sto[:nt, 256 + DT:TROW], 0.0)
            nc.sync.dma_start(out=outs["Tsh"][t * 128:t * 128 + nt, :], in_=sto[:nt, :])


@with_exitstack
def build_phase_b(ctx, tc, outs, ins, cfg):
    nc = tc.nc
    NW = cfg["NW"]
    DO, RK = cfg["DOUT"], cfg["RANK"]
    DT = DO + RK
    BL, BH = cfg["BL"], cfg["BH"]
    woff = cfg["woff"]
    NB = cfg["NB"]
    HI_BASE = cfg["HI_BASE"]
    T_ROWS = cfg["T_ROWS"]
    GROUP = cfg.get("GROUP", 12)
    NBMAX = int(max(BL[w] + BH[w] for w in range(NW)))

    cpool = ctx.enter_context(tc.tile_pool(name="const", bufs=1))
    gpool = ctx.enter_context(tc.tile_pool(name="g", bufs=3 if SPLIT_BF16 else 2))
    anpool = ctx.enter_context(tc.tile_pool(name="an", bufs=2))
    spool = ctx.enter_context(tc.tile_pool(name="s", bufs=2))
    hpool = ctx.enter_context(tc.tile_pool(name="h", bufs=2))
    epool = ctx.enter_context(tc.tile_pool(name="e", bufs=2))
    rpool = ctx.enter_context(tc.tile_pool(name="r", bufs=2))
    accp = ctx.enter_context(tc.tile_pool(name="acc", bufs=4 if SPLIT_BF16 else 2,
                                          space="PSUM"))
    tpp = ctx.enter_context(tc.tile_pool(name="tp", bufs=2, space="PSUM"))
    vpp = ctx.enter_context(tc.tile_pool(name="vp", bufs=2, space="PSUM"))

    idxs = cpool.tile([128, NB * 8], I16)
    nc.sync.dma_start(out=idxs[:], in_=ins["idx16w"][:])
    dl = cpool.tile([128, NB], F32)
    nc.sync.dma_start(out=dl[:], in_=ins["dstloc"][:])
    deg = cpool.tile([128, NW], F32)
    nc.sync.dma_start(out=deg[:], in_=ins["deg_in_sh"][:])
    vm = cpool.tile([64, 128], F32)
    nc.sync.dma_start(out=vm[:RK, :DO], in_=ins["vmat"][:])
    ident = cpool.tile([128, 128], F32)
    make_identity(nc, ident[:])
    iota_i = cpool.tile([128, 128], I32)
    nc.gpsimd.iota(iota_i[:], pattern=[[1, 128]], base=0, channel_multiplier=0)
    iota_f = cpool.tile([128, 128], F32)
    nc.vector.tensor_copy(out=iota_f[:], in_=iota_i[:])
    mask = cpool.tile([128, NW], F32)
    nc.vector.tensor_scalar(out=mask[:], in0=deg[:], scalar1=0.0, scalar2=None, op0=ALU.is_gt)
    inv = cpool.tile([128, NW], F32)
    nc.vector.tensor_scalar(out=inv[:], in0=deg[:], scalar1=1.0, scalar2=None, op0=ALU.max)
    nc.scalar.activation(out=inv[:], in_=inv[:], func=AF.Sqrt)
    nc.vector.reciprocal(out=inv[:], in_=inv[:])

    TLO = ins["T"][0:HI_BASE, :]
    THI = ins["T"][HI_BASE:T_ROWS, :] if any(BH[w] > 0 for w in range(NW)) else None

    out_r = outs["out_sh"].rearrange("(w p) f -> p w f", p=128)

    w = 0
    while w < NW:
        ws = list(range(w, min(w + GROUP, NW)))
        nwg = len(ws)
        g0 = ws[0]
        H = hpool.tile([128, nwg, 256], F32, tag="H")
        for j, wi in enumerate(ws):
            nb = int(BL[wi] + BH[wi])
            b0 = int(woff[wi])
            nl, nh = int(BL[wi]) * 128, int(BH[wi]) * 128
            TW = TROW if SPLIT_BF16 else DT  # T row width in elements
            G = gpool.tile([128, NBMAX, TW], BF16 if SPLIT_BF16 else F32, tag="G")

            def _gather(tbl, blk_lo, blk_hi):
                step = GATHER_CHUNK if GATHER_CHUNK > 0 else (blk_hi - blk_lo)
                for s in range(blk_lo, blk_hi, step):
                    e = min(s + step, blk_hi)
                    ni = (e - s) * 128
                    nc.gpsimd.dma_gather(
                        out_ap=G[:, s:e, :], in_ap=tbl,
                        idxs_ap=idxs[:, (b0 + s) * 8:(b0 + e) * 8],
                        num_idxs=ni, num_idxs_reg=ni, elem_size=TW,
                        single_packet=False)

            _gather(TLO, 0, int(BL[wi]))
            if BH[wi] > 0:
                _gather(THI, int(BL[wi]), nb)
            if SPLIT_BF16:
                # G cols: [fs_hi | absv_hi | neg | fs_lo | absv_lo | pad]
                S = spool.tile([128, NBMAX, 128], BF16, tag="S")
                nc.vector.tensor_tensor(
                    out=S[:, :nb, :],
                    in0=dl[:, b0:b0 + nb].unsqueeze(2).to_broadcast([128, nb, 128]),
                    in1=iota_f[:].unsqueeze(1).to_broadcast([128, nb, 128]),
                    op=ALU.is_equal)
                ps1 = accp.tile([128, 256], F32, tag="acc1")
                for b in range(nb):
                    nc.tensor.matmul(out=ps1[:], lhsT=S[:, b, :], rhs=G[:, b, 0:256],
                                     start=(b == 0), stop=False)
                    nc.tensor.matmul(out=ps1[:, 0:DT], lhsT=S[:, b, :],
                                     rhs=G[:, b, 256:256 + DT],
                                     start=False, stop=(b == nb - 1))
                if j % 2 == 0:
                    nc.vector.tensor_copy(out=H[:, j, :], in_=ps1[:])
                else:
                    nc.scalar.activation(out=H[:, j, :], in_=ps1[:], func=AF.Copy)
            else:
                AN = anpool.tile([128, NBMAX, 128], F32, tag="AN")
                nc.vector.tensor_scalar(out=AN[:, :nb, 0:RK].bitcast(I32),
                                        in0=G[:, :nb, DO:DT].bitcast(I32),
                                        scalar1=MASK_ABS, scalar2=None, op0=ALU.bitwise_and)
                nc.vector.tensor_scalar(out=AN[:, :nb, RK:128], in0=G[:, :nb, DO:DT],
                                        scalar1=0.0, scalar2=None, op0=ALU.is_lt)
                S = spool.tile([128, NBMAX, 128], F32, tag="S")
                nc.vector.tensor_tensor(
                    out=S[:, :nb, :],
                    in0=dl[:, b0:b0 + nb].unsqueeze(2).to_broadcast([128, nb, 128]),
                    in1=iota_f[:].unsqueeze(1).to_broadcast([128, nb, 128]),
                    op=ALU.is_equal)
                ps1 = accp.tile([128, DO], F32, tag="acc1")
                ps2 = accp.tile([128, 128], F32, tag="acc2")
                for b in range(nb):
                    nc.tensor.matmul(out=ps1[:], lhsT=S[:, b, :], rhs=G[:, b, 0:DO],
                                     start=(b == 0), stop=(b == nb - 1))
                    nc.tensor.matmul(out=ps2[:], lhsT=S[:, b, :], rhs=AN[:, b, :],
                                     start=(b == 0), stop=(b == nb - 1))
                if j % 2 == 0:
                    nc.vector.tensor_copy(out=H[:, j, 0:DO], in_=ps1[:])
                    nc.scalar.activation(out=H[:, j, DO:256], in_=ps2[:], func=AF.Copy)
                else:
                    nc.scalar.activation(out=H[:, j, 0:DO], in_=ps1[:], func=AF.Copy)
                    nc.vector.tensor_copy(out=H[:, j, DO:256], in_=ps2[:])
        # epilogue for the group
        pi = epool.tile([128, GROUP, RK], I32, tag="pi")
        nc.vector.tensor_copy(out=pi[:, :nwg, :], in_=H[:, :, DO + RK:256])
        nc.vector.tensor_scalar(out=pi[:, :nwg, :], in0=pi[:, :nwg, :], scalar1=1,
                                scalar2=None, op0=ALU.bitwise_and)
        sg = epool.tile([128, GROUP, RK], F32, tag="sg")
        nc.vector.tensor_copy(out=sg[:, :nwg, :], in_=pi[:, :nwg, :])
        nc.vector.tensor_scalar(out=sg[:, :nwg, :], in0=sg[:, :nwg, :], scalar1=-2.0,
                                scalar2=1.0, op0=ALU.mult, op1=ALU.add)
        ex = epool.tile([128, GROUP, RK], F32, tag="ex")
        nc.scalar.activation(out=ex[:, :nwg, :], in_=H[:, :, DO:DO + RK], func=AF.Exp, scale=-1.0)
        nc.vector.tensor_tensor(out=ex[:, :nwg, :], in0=ex[:, :nwg, :], in1=sg[:, :nwg, :], op=ALU.mult)
        nc.vector.tensor_tensor(
            out=ex[:, :nwg, :], in0=ex[:, :nwg, :],
            in1=mask[:, g0:g0 + nwg].unsqueeze(2).to_broadcast([128, nwg, RK]),
            op=ALU.mult)
        RST = rpool.tile([128, GROUP, DO], F32, tag="RST")
        for j, wi in enumerate(ws):
            tp = tpp.tile([64, 128], F32, tag="tp")
            nc.tensor.transpose(out=tp[:RK, :], in_=ex[:, j, :], identity=ident[:])
            hpT = epool.tile([64, 128], F32, tag="hpT")
            nc.vector.tensor_copy(out=hpT[:RK, :], in_=tp[:RK, :])
            vp = vpp.tile([128, DO], F32, tag="vpp")
            nc.tensor.matmul(out=vp[:], lhsT=hpT[:RK, :], rhs=vm[:RK, :DO], start=True, stop=True)
            nc.vector.tensor_tensor(out=RST[:, j, :], in0=H[:, j, 0:DO], in1=vp[:], op=ALU.add)
        nc.vector.tensor_tensor(
            out=RST[:, :nwg, :], in0=RST[:, :nwg, :],
            in1=inv[:, g0:g0 + nwg].unsqueeze(2).to_broadcast([128, nwg, DO]),
            op=ALU.mult)
        nc.sync.dma_start(out=out_r[:, g0:g0 + nwg, :], in_=RST[:, :nwg, :])
        w += GROUP


# ---------------- SPMD drivers ----------------

def _new_nc(n_cores):
    return bacc.Bacc("TRN2", target_bir_lowering=False, debug=False,
                     enable_asserts=False, num_devices=n_cores)


def _build_a(dm):
    nc = _new_nc(dm["M"])
    DIN, NSH, NW = dm["DIN"], dm["NSH"], dm["NW"]
    DT = dm["DOUT"] + dm["RANK"]
    ins = dict(
        featT=nc.dram_tensor("featT", [DIN, NSH], F32, kind="ExternalInput").ap(),
        Wcat=nc.dram_tensor("Wcat", [DIN, DT], F32, kind="ExternalInput").ap(),
        brow=nc.dram_tensor("brow", [1, DT], F32, kind="ExternalInput").ap(),
        deg_out_sh=nc.dram_tensor("deg_out_sh", [128, NW], F32, kind="ExternalInput").ap(),
    )
    if SPLIT_BF16:
        outs = dict(Tsh=nc.dram_tensor("Tsh", [NSH, TROW], BF16, kind="ExternalOutput").ap())
    else:
        outs = dict(Tsh=nc.dram_tensor("Tsh", [NSH, DT], F32, kind="ExternalOutput").ap())
    cfg = dict(NSH=NSH, NW=NW, DIN=DIN, DOUT=dm["DOUT"], RANK=dm["RANK"])
    with tile.TileContext(nc) as tc:
        build_phase_a(tc, outs, ins, cfg)
    nc.compile()
    return nc


def _build_b(dm, pp, group=12):
    nc = _new_nc(dm["M"])
    NW, NB = dm["NW"], pp["NB"]
    DT = dm["DOUT"] + dm["RANK"]
    TW_ = TROW if SPLIT_BF16 else DT
    ins = dict(
        T=nc.dram_tensor("T", [dm["T_ROWS"], TW_], BF16 if SPLIT_BF16 else F32,
                         kind="ExternalInput").ap(),
        idx16w=nc.dram_tensor("idx16w", [128, NB * 8], I16, kind="ExternalInput").ap(),
        dstloc=nc.dram_tensor("dstloc", [128, NB], F32, kind="ExternalInput").ap(),
        deg_in_sh=nc.dram_tensor("deg_in_sh", [128, NW], F32, kind="ExternalInput").ap(),
        vmat=nc.dram_tensor("vmat", [dm["RANK"], dm["DOUT"]], F32, kind="ExternalInput").ap(),
    )
    outs = dict(out_sh=nc.dram_tensor("out_sh", [NW * 128, dm["DOUT"]], F32,
                                      kind="ExternalOutput").ap())
    cfg = dict(NW=NW, DOUT=dm["DOUT"], RANK=dm["RANK"], BL=pp["BL"], BH=pp["BH"],
               woff=pp["woff"], NB=NB, HI_BASE=dm["HI_BASE"], T_ROWS=dm["T_ROWS"],
               GROUP=group)
    with tile.TileContext(nc) as tc:
        build_phase_b(tc, outs, ins, cfg)
    nc.compile()
    return nc


def run_all(feat, w1, w2, v, src, dst, trace=False, tmpdir_a=None, tmpdir_b=None):
    """Returns (output, info dict with per-launch BassKernelResults)."""
    dm = make_dims(N=feat.shape[0], E=src.shape[0], DIN=feat.shape[1],
                   DOUT=w1.shape[1], RANK=v.shape[0])
    pp = preprocess(src, dst, dm)
    M, NSH = dm["M"], dm["NSH"]

    ains = host_phase_a_inputs(feat, w1, w2, pp, dm)
    nc_a = _build_a(dm)
    ra = bass_utils.run_bass_kernel_spmd(nc_a, ains, list(range(M)), trace=trace,
                                         tmpdir=tmpdir_a)
    shards = [ra.results[k]["Tsh"] for k in range(M)]
    T = assemble_T(shards, dm)

    bins = host_phase_b_inputs(T, pp, v, dm)
    nc_b = _build_b(dm, pp)
    rb = bass_utils.run_bass_kernel_spmd(nc_b, bins, list(range(M)), trace=trace,
                                         tmpdir=tmpdir_b)
    out = np.concatenate([rb.results[k]["out_sh"][:NSH] for k in range(M)], axis=0)
    return out.astype(np.float32), dict(ra=ra, rb=rb, dm=dm, pp=pp)


def kernel(feat, w1, w2, v, src, dst):
    feat = np.asarray(feat, np.float32)
    w1 = np.asarray(w1, np.float32)
    w2 = np.asarray(w2, np.float32)
    v = np.asarray(v, np.float32)
    src = np.asarray(src)
    dst = np.asarray(dst)
    out, _ = run_all(feat, w1, w2, v, src, dst, trace=False)
    return out



# revision 4
# speedup vs baseline: 1.0377x; 1.0377x over previous
"""Trainium2 Bass kernel for nn_DGLGraphConv (graph conv with sum- and product-reduce).

Strategy (8 NeuronCores, SPMD, two launches):
  Launch A (node-sharded, 6250 nodes/core): per-node table, bf16, 256 cols:
      T[n] = [ (feat@w1)*s_out (128) | |log|tanh|| (64) | neg indicator (64) ]
  Host: concat T shards (pure relayout; row 0 / tail rows are zeros used as gather padding).
  Launch B (dst-sharded by edge partitioning): per core, 49 windows x 128 dst slots.
      Edges sorted by (dst window, src half, src), padded to 128-edge blocks with
      block counts maxed over cores so all cores run one program. Per block:
      dma_gather 128 rows of T (512B each), one-hot S matmul accumulates
      [sum(fs) | sum|ln| | count(neg)] per dst slot in PSUM; epilogue computes
      sign via parity, exp, mask, @v, and the in-degree scaling.

Host does integer index prep (bincount/sort/pad) and layout-only transforms;
all floating-point math runs on device.
"""
import sys
from contextlib import ExitStack

import numpy as np

for _p in ("/opt/trn_rl_repo",):
    if _p not in sys.path:
        sys.path.insert(0, _p)

import concourse.bass as bass
import concourse.mybir as mybir
import concourse.tile as tile
from concourse import bacc, bass_utils
from concourse._compat import with_exitstack
from concourse.masks import make_identity

F32 = mybir.dt.float32
I32 = mybir.dt.int32
I16 = mybir.dt.int16
BF16 = mybir.dt.bfloat16
AF = mybir.ActivationFunctionType
ALU = mybir.AluOpType

MASK_ABS = 0x7FFFFFFF
MASK_SGN = -0x80000000

# T row layout (bf16, 256 cols = 512B):
#   [fs(128) | absl(64) | neg(64)]
TROW = 256
import os as _os
# Max 128-edge blocks per dma_gather call. Full-window calls (~2200 indices)
# die on HW (descriptor-ring overflow); 8 blocks = 1024 descriptors is safe.
GATHER_CHUNK = int(_os.environ.get("GCH", "8"))
SINGLE_PACKET = _os.environ.get("SPKT", "0") == "1"
NQUEUE = int(_os.environ.get("NQ", "1"))  # rotate gather calls over SWDGE queues


# ---------------- host-side prep (integer/layout only) ----------------

def make_dims(N=50000, E=800000, DIN=256, DOUT=128, RANK=64, M=8, LO_MAX=32766):
    LO_MAX = min(LO_MAX, N - 1)
    NSH = N // M
    W = 128
    NW = (NSH + W - 1) // W
    T_ROWS = N + 4
    HI_BASE = LO_MAX + 2
    NHI = T_ROWS - HI_BASE
    return dict(N=N, E=E, DIN=DIN, DOUT=DOUT, RANK=RANK, M=M, NSH=NSH, W=W, NW=NW,
                LO_MAX=LO_MAX, T_ROWS=T_ROWS, HI_BASE=HI_BASE, NHI=NHI,
                HI_PAD_IDX=NHI - 2)


def preprocess(src, dst, dm):
    N, E, M, NSH, W, NW = dm["N"], dm["E"], dm["M"], dm["NSH"], dm["W"], dm["NW"]
    LO_MAX, HI_PAD_IDX = dm["LO_MAX"], dm["HI_PAD_IDX"]
    src = np.asarray(src).astype(np.int64)
    dst = np.asarray(dst).astype(np.int64)
    deg_out = np.bincount(src, minlength=N).astype(np.float32)
    deg_in = np.bincount(dst, minlength=N).astype(np.float32)

    order = np.lexsort((src, dst))
    s_srt, d_srt = src[order], dst[order]
    core_of = d_srt // NSH
    win_of = (d_srt - core_of * NSH) // W

    is_hi = s_srt > LO_MAX
    nlo = np.zeros((M, NW), np.int64)
    nhi = np.zeros((M, NW), np.int64)
    np.add.at(nlo, (core_of[~is_hi], win_of[~is_hi]), 1)
    np.add.at(nhi, (core_of[is_hi], win_of[is_hi]), 1)

    BL = np.maximum(1, -(-nlo.max(axis=0) // 128))
    BH = -(-nhi.max(axis=0) // 128)
    BT = BL + BH
    NB = int(BT.sum())

    idx16 = np.zeros((M, NB * 128), np.int16)
    dstloc = np.zeros((M, 128, NB), np.float32)
    key_half = is_hi.astype(np.int64)
    order2 = np.lexsort((s_srt, key_half, win_of, core_of))
    s2, d2 = s_srt[order2], d_srt[order2]
    c2, w2_, h2 = core_of[order2], win_of[order2], key_half[order2]

    woff = np.concatenate([[0], np.cumsum(BT)])
    counts = np.zeros((M, NW, 2), np.int64)
    np.add.at(counts, (c2, w2_, h2), 1)
    ptr = 0
    for k in range(M):
        for w in range(NW):
            cl, ch = counts[k, w, 0], counts[k, w, 1]
            base_blk = woff[w]
            seg = slice(ptr, ptr + cl)
            pos = np.arange(cl)
            idx16[k, base_blk * 128 + pos] = (s2[seg] + 1).astype(np.int16)
            dstloc[k, pos % 128, base_blk + pos // 128] = (d2[seg] - k * NSH - w * W).astype(np.float32)
            ptr += cl
            base_blk_h = woff[w] + BL[w]
            seg = slice(ptr, ptr + ch)
            pos = np.arange(ch)
            idx16[k, base_blk_h * 128 + pos] = (s2[seg] - LO_MAX - 1).astype(np.int16)
            dstloc[k, pos % 128, base_blk_h + pos // 128] = (d2[seg] - k * NSH - w * W).astype(np.float32)
            ptr += ch
            if ch < BH[w] * 128:
                pad = np.arange(ch, BH[w] * 128)
                idx16[k, base_blk_h * 128 + pad] = HI_PAD_IDX
    assert ptr == E

    idx16_wrapped = np.tile(idx16.reshape(M, -1, 16).transpose(0, 2, 1), (1, 8, 1)).copy()

    def shard_deg(deg):
        out = np.zeros((M, 128, NW), np.float32)
        for k in range(M):
            d = deg[k * NSH:(k + 1) * NSH]
            d = np.concatenate([d, np.zeros(NW * W - NSH, np.float32)])
            out[k] = d.reshape(NW, W).T
        return out

    return dict(BL=BL, BH=BH, BT=BT, NB=NB, idx16_wrapped=idx16_wrapped, dstloc=dstloc,
                deg_in_sh=shard_deg(deg_in), deg_out_sh=shard_deg(deg_out), woff=woff)


def host_phase_a_inputs(feat, w1, w2, pp, dm):
    M, NSH, DIN, DOUT, RANK = dm["M"], dm["NSH"], dm["DIN"], dm["DOUT"], dm["RANK"]
    featT = np.ascontiguousarray(np.asarray(feat).T)
    Wcat = np.ascontiguousarray(np.concatenate([w1, w2[:DIN]], axis=1), dtype=np.float32)
    brow = np.zeros((1, DOUT + RANK), np.float32)
    brow[0, DOUT:] = w2[DIN]
    return [dict(featT=np.ascontiguousarray(featT[:, k * NSH:(k + 1) * NSH]),
                 Wcat=Wcat, brow=brow, deg_out_sh=pp["deg_out_sh"][k])
            for k in range(M)]


def assemble_T(shards, dm):
    width = shards[0].shape[1]
    T = np.zeros((dm["T_ROWS"], width), shards[0].dtype)
    for k in range(dm["M"]):
        T[1 + k * dm["NSH"]:1 + (k + 1) * dm["NSH"]] = shards[k][:dm["NSH"]]
    return T


def host_phase_b_inputs(T, pp, vmat, dm):
    return [dict(T=T, idx16w=pp["idx16_wrapped"][k], dstloc=pp["dstloc"][k],
                 deg_in_sh=pp["deg_in_sh"][k], vmat=np.ascontiguousarray(vmat, np.float32))
            for k in range(dm["M"])]


# ---------------- device kernels ----------------

@with_exitstack
def build_phase_a(ctx, tc, outs, ins, cfg):
    nc = tc.nc
    NSH, NW = cfg["NSH"], cfg["NW"]
    DIN = cfg["DIN"]
    DO, RK = cfg["DOUT"], cfg["RANK"]
    DT = DO + RK
    KC = DIN // 128

    cpool = ctx.enter_context(tc.tile_pool(name="const", bufs=1))
    wpool = ctx.enter_context(tc.tile_pool(name="work", bufs=4))
    ppool = ctx.enter_context(tc.tile_pool(name="psum", bufs=4, space="PSUM"))

    featT = cpool.tile([128, KC, NSH], F32)
    nc.sync.dma_start(out=featT[:],
                      in_=ins["featT"].rearrange("(c p) n -> p c n", p=128))
    Wsb = cpool.tile([128, KC, DT], F32)
    nc.sync.dma_start(out=Wsb[:],
                      in_=ins["Wcat"].rearrange("(c p) n -> p c n", p=128))
    bsb = cpool.tile([1, DT], F32)
    nc.sync.dma_start(out=bsb[:], in_=ins["brow"][:])
    ones = cpool.tile([1, 128], F32)
    nc.vector.memset(ones[:], 1.0)
    # replicate bias across partitions once: brep = ones.T @ bsb
    bps = ppool.tile([128, DT], F32, tag="bps")
    nc.tensor.matmul(out=bps[:], lhsT=ones[:], rhs=bsb[:], start=True, stop=True)
    brep = cpool.tile([128, DT], F32)
    nc.vector.tensor_copy(out=brep[:], in_=bps[:])
    deg = cpool.tile([128, NW], F32)
    nc.sync.dma_start(out=deg[:], in_=ins["deg_out_sh"][:])
    s_all = cpool.tile([128, NW], F32)
    nc.vector.tensor_scalar(out=s_all[:], in0=deg[:], scalar1=1.0, scalar2=None, op0=ALU.max)
    nc.scalar.activation(out=s_all[:], in_=s_all[:], func=AF.Sqrt)
    nc.vector.reciprocal(out=s_all[:], in_=s_all[:])

    for t in range(NW):
        nt = min(128, NSH - t * 128)
        ps = ppool.tile([128, DT], F32, tag="ps")
        for c in range(KC):
            nc.tensor.matmul(out=ps[:nt, :], lhsT=featT[:, c, t * 128:t * 128 + nt],
                             rhs=Wsb[:, c, :], start=(c == 0), stop=(c == KC - 1))
        st = wpool.tile([128, DT], F32, tag="st")
        nc.vector.tensor_scalar(out=st[:nt, :], in0=ps[:nt, :], scalar1=s_all[:nt, t:t + 1],
                                scalar2=None, op0=ALU.mult)
        nc.vector.tensor_tensor(out=st[:nt, :], in0=st[:nt, :], in1=brep[:nt, :], op=ALU.add)
        tnh = wpool.tile([128, RK], F32, tag="tnh")
        nc.scalar.activation(out=tnh[:nt, :], in_=st[:nt, DO:DT], func=AF.Tanh)
        ab = wpool.tile([128, RK], F32, tag="ab")
        nc.vector.tensor_scalar(out=ab[:nt, :].bitcast(I32), in0=tnh[:nt, :].bitcast(I32),
                                scalar1=MASK_ABS, scalar2=None, op0=ALU.bitwise_and)
        ln = wpool.tile([128, RK], F32, tag="ln")
        nc.scalar.activation(out=ln[:nt, :], in_=ab[:nt, :], func=AF.Ln)
        # ln<=-1e-7 (strictly negative), then |ln| into st cols DO:DT
        nc.vector.tensor_scalar(out=ln[:nt, :], in0=ln[:nt, :], scalar1=-1e-7, scalar2=None, op0=ALU.min)
        nc.vector.tensor_scalar(out=st[:nt, DO:DT].bitcast(I32), in0=ln[:nt, :].bitcast(I32),
                                scalar1=MASK_ABS, scalar2=None, op0=ALU.bitwise_and)
        sto = wpool.tile([128, TROW], BF16, tag="sto")
        # neg indicator from tanh sign
        nc.vector.tensor_scalar(out=sto[:nt, DT:TROW], in0=tnh[:nt, :],
                                scalar1=0.0, scalar2=None, op0=ALU.is_lt)
        nc.vector.tensor_copy(out=sto[:nt, 0:DT], in_=st[:nt, :])
        nc.sync.dma_start(out=outs["Tsh"][t * 128:t * 128 + nt, :], in_=sto[:nt, :])


@with_exitstack
def build_phase_b(ctx, tc, outs, ins, cfg):
    nc = tc.nc
    NW = cfg["NW"]
    DO, RK = cfg["DOUT"], cfg["RANK"]
    DT = DO + RK
    BL, BH = cfg["BL"], cfg["BH"]
    woff = cfg["woff"]
    NB = cfg["NB"]
    HI_BASE = cfg["HI_BASE"]
    T_ROWS = cfg["T_ROWS"]
    GROUP = cfg.get("GROUP", 12)
    NBMAX = int(max(BL[w] + BH[w] for w in range(NW)))

    cpool = ctx.enter_context(tc.tile_pool(name="const", bufs=1))
    gpool = ctx.enter_context(tc.tile_pool(name="g", bufs=3))
    spool = ctx.enter_context(tc.tile_pool(name="s", bufs=2))
    hpool = ctx.enter_context(tc.tile_pool(name="h", bufs=2))
    epool = ctx.enter_context(tc.tile_pool(name="e", bufs=2))
    rpool = ctx.enter_context(tc.tile_pool(name="r", bufs=2))
    accp = ctx.enter_context(tc.tile_pool(name="acc", bufs=2, space="PSUM"))
    tpp = ctx.enter_context(tc.tile_pool(name="tp", bufs=2, space="PSUM"))
    vpp = ctx.enter_context(tc.tile_pool(name="vp", bufs=2, space="PSUM"))

    idxs = cpool.tile([128, NB * 8], I16)
    nc.sync.dma_start(out=idxs[:], in_=ins["idx16w"][:])
    dl = cpool.tile([128, NB], F32)
    nc.sync.dma_start(out=dl[:], in_=ins["dstloc"][:])
    deg = cpool.tile([128, NW], F32)
    nc.sync.dma_start(out=deg[:], in_=ins["deg_in_sh"][:])
    vm = cpool.tile([64, 128], F32)
    nc.sync.dma_start(out=vm[:RK, :DO], in_=ins["vmat"][:])
    ident = cpool.tile([128, 128], F32)
    make_identity(nc, ident[:])
    iota_i = cpool.tile([128, 128], I32)
    nc.gpsimd.iota(iota_i[:], pattern=[[1, 128]], base=0, channel_multiplier=0)
    iota_f = cpool.tile([128, 128], F32)
    nc.vector.tensor_copy(out=iota_f[:], in_=iota_i[:])
    mask = cpool.tile([128, NW], F32)
    nc.vector.tensor_scalar(out=mask[:], in0=deg[:], scalar1=0.0, scalar2=None, op0=ALU.is_gt)
    inv = cpool.tile([128, NW], F32)
    nc.vector.tensor_scalar(out=inv[:], in0=deg[:], scalar1=1.0, scalar2=None, op0=ALU.max)
    nc.scalar.activation(out=inv[:], in_=inv[:], func=AF.Sqrt)
    nc.vector.reciprocal(out=inv[:], in_=inv[:])

    TLO = ins["T"][0:HI_BASE, :]
    THI = ins["T"][HI_BASE:T_ROWS, :] if any(BH[w] > 0 for w in range(NW)) else None

    out_r = outs["out_sh"].rearrange("(w p) f -> p w f", p=128)

    qn = [0]

    w = 0
    while w < NW:
        ws = list(range(w, min(w + GROUP, NW)))
        nwg = len(ws)
        g0 = ws[0]
        H = hpool.tile([128, nwg, 256], F32, tag="H")
        for j, wi in enumerate(ws):
            nb = int(BL[wi] + BH[wi])
            b0 = int(woff[wi])
            G = gpool.tile([128, NBMAX, TROW], BF16, tag="G")

            def _gather(tbl, blk_lo, blk_hi):
                step = GATHER_CHUNK if GATHER_CHUNK > 0 else (blk_hi - blk_lo)
                for s in range(blk_lo, blk_hi, step):
                    e = min(s + step, blk_hi)
                    ni = (e - s) * 128
                    nc.gpsimd.dma_gather(
                        out_ap=G[:, s:e, :], in_ap=tbl,
                        idxs_ap=idxs[:, (b0 + s) * 8:(b0 + e) * 8],
                        num_idxs=ni, num_idxs_reg=ni, elem_size=TROW,
                        single_packet=SINGLE_PACKET,
                        queue_num=qn[0] % NQUEUE)
                    qn[0] += 1

            _gather(TLO, 0, int(BL[wi]))
            if BH[wi] > 0:
                _gather(THI, int(BL[wi]), nb)
            S = spool.tile([128, NBMAX, 128], BF16, tag="S")
            nc.vector.tensor_tensor(
                out=S[:, :nb, :],
                in0=dl[:, b0:b0 + nb].unsqueeze(2).to_broadcast([128, nb, 128]),
                in1=iota_f[:].unsqueeze(1).to_broadcast([128, nb, 128]),
                op=ALU.is_equal)
            ps1 = accp.tile([128, 256], F32, tag="acc1")
            for b in range(nb):
                nc.tensor.matmul(out=ps1[:], lhsT=S[:, b, :], rhs=G[:, b, :],
                                 start=(b == 0), stop=(b == nb - 1))
            if j % 2 == 0:
                nc.vector.tensor_copy(out=H[:, j, :], in_=ps1[:])
            else:
                nc.scalar.activation(out=H[:, j, :], in_=ps1[:], func=AF.Copy)
        # epilogue for the group
        pi = epool.tile([128, GROUP, RK], I32, tag="pi")
        nc.vector.tensor_copy(out=pi[:, :nwg, :], in_=H[:, :, DO + RK:256])
        nc.vector.tensor_scalar(out=pi[:, :nwg, :], in0=pi[:, :nwg, :], scalar1=1,
                                scalar2=None, op0=ALU.bitwise_and)
        sg = epool.tile([128, GROUP, RK], F32, tag="sg")
        nc.vector.tensor_copy(out=sg[:, :nwg, :], in_=pi[:, :nwg, :])
        nc.vector.tensor_scalar(out=sg[:, :nwg, :], in0=sg[:, :nwg, :], scalar1=-2.0,
                                scalar2=1.0, op0=ALU.mult, op1=ALU.add)
        ex = epool.tile([128, GROUP, RK], F32, tag="ex")
        nc.scalar.activation(out=ex[:, :nwg, :], in_=H[:, :, DO:DO + RK], func=AF.Exp, scale=-1.0)
        nc.vector.tensor_tensor(out=ex[:, :nwg, :], in0=ex[:, :nwg, :], in1=sg[:, :nwg, :], op=ALU.mult)
        nc.vector.tensor_tensor(
            out=ex[:, :nwg, :], in0=ex[:, :nwg, :],
            in1=mask[:, g0:g0 + nwg].unsqueeze(2).to_broadcast([128, nwg, RK]),
            op=ALU.mult)
        RST = rpool.tile([128, GROUP, DO], F32, tag="RST")
        for j, wi in enumerate(ws):
            tp = tpp.tile([64, 128], F32, tag="tp")
            nc.tensor.transpose(out=tp[:RK, :], in_=ex[:, j, :], identity=ident[:])
            hpT = epool.tile([64, 128], F32, tag="hpT")
            nc.vector.tensor_copy(out=hpT[:RK, :], in_=tp[:RK, :])
            vp = vpp.tile([128, DO], F32, tag="vpp")
            nc.tensor.matmul(out=vp[:], lhsT=hpT[:RK, :], rhs=vm[:RK, :DO], start=True, stop=True)
            nc.vector.tensor_tensor(out=RST[:, j, :], in0=H[:, j, 0:DO], in1=vp[:], op=ALU.add)
        nc.vector.tensor_tensor(
            out=RST[:, :nwg, :], in0=RST[:, :nwg, :],
            in1=inv[:, g0:g0 + nwg].unsqueeze(2).to_broadcast([128, nwg, DO]),
            op=ALU.mult)
        nc.sync.dma_start(out=out_r[:, g0:g0 + nwg, :], in_=RST[:, :nwg, :])
        w += GROUP


# ---------------- SPMD drivers ----------------

def _new_nc(n_cores):
    return bacc.Bacc("TRN2", target_bir_lowering=False, debug=False,
                     enable_asserts=False, num_devices=n_cores)


def _build_a(dm):
    nc = _new_nc(dm["M"])
    DIN, NSH, NW = dm["DIN"], dm["NSH"], dm["NW"]
    DT = dm["DOUT"] + dm["RANK"]
    ins = dict(
        featT=nc.dram_tensor("featT", [DIN, NSH], F32, kind="ExternalInput").ap(),
        Wcat=nc.dram_tensor("Wcat", [DIN, DT], F32, kind="ExternalInput").ap(),
        brow=nc.dram_tensor("brow", [1, DT], F32, kind="ExternalInput").ap(),
        deg_out_sh=nc.dram_tensor("deg_out_sh", [128, NW], F32, kind="ExternalInput").ap(),
    )
    outs = dict(Tsh=nc.dram_tensor("Tsh", [NSH, TROW], BF16, kind="ExternalOutput").ap())
    cfg = dict(NSH=NSH, NW=NW, DIN=DIN, DOUT=dm["DOUT"], RANK=dm["RANK"])
    with tile.TileContext(nc) as tc:
        build_phase_a(tc, outs, ins, cfg)
    nc.compile()
    return nc


def _build_b(dm, pp, group=12):
    nc = _new_nc(dm["M"])
    NW, NB = dm["NW"], pp["NB"]
    ins = dict(
        T=nc.dram_tensor("T", [dm["T_ROWS"], TROW], BF16, kind="ExternalInput").ap(),
        idx16w=nc.dram_tensor("idx16w", [128, NB * 8], I16, kind="ExternalInput").ap(),
        dstloc=nc.dram_tensor("dstloc", [128, NB], F32, kind="ExternalInput").ap(),
        deg_in_sh=nc.dram_tensor("deg_in_sh", [128, NW], F32, kind="ExternalInput").ap(),
        vmat=nc.dram_tensor("vmat", [dm["RANK"], dm["DOUT"]], F32, kind="ExternalInput").ap(),
    )
    outs = dict(out_sh=nc.dram_tensor("out_sh", [NW * 128, dm["DOUT"]], F32,
                                      kind="ExternalOutput").ap())
    cfg = dict(NW=NW, DOUT=dm["DOUT"], RANK=dm["RANK"], BL=pp["BL"], BH=pp["BH"],
               woff=pp["woff"], NB=NB, HI_BASE=dm["HI_BASE"], T_ROWS=dm["T_ROWS"],
               GROUP=group)
    with tile.TileContext(nc) as tc:
        build_phase_b(tc, outs, ins, cfg)
    nc.compile()
    return nc


def run_all(feat, w1, w2, v, src, dst, trace=False, tmpdir_a=None, tmpdir_b=None):
    """Returns (output, info dict with per-launch BassKernelResults)."""
    dm = make_dims(N=feat.shape[0], E=src.shape[0], DIN=feat.shape[1],
                   DOUT=w1.shape[1], RANK=v.shape[0])
    pp = preprocess(src, dst, dm)
    M, NSH = dm["M"], dm["NSH"]

    ains = host_phase_a_inputs(feat, w1, w2, pp, dm)
    nc_a = _build_a(dm)
    ra = bass_utils.run_bass_kernel_spmd(nc_a, ains, list(range(M)), trace=trace,
                                         tmpdir=tmpdir_a)
    shards = [ra.results[k]["Tsh"] for k in range(M)]
    T = assemble_T(shards, dm)

    bins = host_phase_b_inputs(T, pp, v, dm)
    nc_b = _build_b(dm, pp)
    rb = bass_utils.run_bass_kernel_spmd(nc_b, bins, list(range(M)), trace=trace,
                                         tmpdir=tmpdir_b)
    out = np.concatenate([rb.results[k]["out_sh"][:NSH] for k in range(M)], axis=0)
    return out.astype(np.float32), dict(ra=ra, rb=rb, dm=dm, pp=pp)


def kernel(feat, w1, w2, v, src, dst):
    feat = np.asarray(feat, np.float32)
    w1 = np.asarray(w1, np.float32)
    w2 = np.asarray(w2, np.float32)
    v = np.asarray(v, np.float32)
    src = np.asarray(src)
    dst = np.asarray(dst)
    out, _ = run_all(feat, w1, w2, v, src, dst, trace=False)
    return out


# revision 14
# speedup vs baseline: 1.1352x; 1.0940x over previous
"""Trainium2 Bass kernel for nn_DGLGraphConv (graph conv with sum- and product-reduce).

Strategy (8 NeuronCores, SPMD, two launches):
  Launch A (node-sharded, 6250 nodes/core): per-node table, bf16, 256 cols:
      T[n] = [ (feat@w1)*s_out (128) | |log|tanh|| (64) | neg indicator (64) ]
  Host: concat T shards (pure relayout; row 0 / tail rows are zeros used as gather padding).
  Launch B (dst-sharded by edge partitioning): per core, 49 windows x 128 dst slots.
      Edges sorted by (dst window, src half, src), padded to 128-edge blocks with
      block counts maxed over cores so all cores run one program. Per block:
      dma_gather 128 rows of T (512B each), one-hot S matmul accumulates
      [sum(fs) | sum|ln| | count(neg)] per dst slot in PSUM; epilogue computes
      sign via parity, exp, mask, @v, and the in-degree scaling.

Host does integer index prep (bincount/sort/pad) and layout-only transforms;
all floating-point math runs on device.
"""
import sys
from contextlib import ExitStack

import numpy as np

for _p in ("/opt/trn_rl_repo",):
    if _p not in sys.path:
        sys.path.insert(0, _p)

import concourse.bass as bass
import concourse.mybir as mybir
import concourse.tile as tile
from concourse import bacc, bass_utils
from concourse._compat import with_exitstack
from concourse.masks import make_identity

F32 = mybir.dt.float32
I32 = mybir.dt.int32
I16 = mybir.dt.int16
BF16 = mybir.dt.bfloat16
AF = mybir.ActivationFunctionType
ALU = mybir.AluOpType

MASK_ABS = 0x7FFFFFFF
MASK_SGN = -0x80000000

# T row layout (bf16, 256 cols = 512B):
#   [fs(128) | absl(64) | neg(64)]
TROW = 256
import os as _os
# Max 128-edge blocks per dma_gather call. Full-window calls (~2200 emitted
# descriptors) die on HW (descriptor-ring overflow); pad slots are emitted as
# trailing -1 (skipped by the ucode), keeping per-call emission ~<=1450.
GATHER_CHUNK = int(_os.environ.get("GCH", "8"))
SINGLE_PACKET = _os.environ.get("SPKT", "0") == "1"
NQUEUE = int(_os.environ.get("NQ", "1"))  # rotate gather calls over SWDGE queues
# windows whose G buffer may be uninitialized SBUF (NaN patterns): gather real
# zero rows for padding there instead of skipping, so 0*NaN never reaches PSUM.
PAD_REAL_WINDOWS = 3


# ---------------- host-side prep (integer/layout only) ----------------

def make_dims(N=50000, E=800000, DIN=256, DOUT=128, RANK=64, M=8, LO_MAX=32766):
    LO_MAX = min(LO_MAX, N - 1)
    NSH = N // M
    W = 128
    NW = (NSH + W - 1) // W
    T_ROWS = N + 4
    HI_BASE = LO_MAX + 2
    NHI = T_ROWS - HI_BASE
    return dict(N=N, E=E, DIN=DIN, DOUT=DOUT, RANK=RANK, M=M, NSH=NSH, W=W, NW=NW,
                LO_MAX=LO_MAX, T_ROWS=T_ROWS, HI_BASE=HI_BASE, NHI=NHI,
                HI_PAD_IDX=NHI - 2)


def preprocess(src, dst, dm):
    N, E, M, NSH, W, NW = dm["N"], dm["E"], dm["M"], dm["NSH"], dm["W"], dm["NW"]
    LO_MAX, HI_PAD_IDX = dm["LO_MAX"], dm["HI_PAD_IDX"]
    src = np.asarray(src).astype(np.int64)
    dst = np.asarray(dst).astype(np.int64)
    deg_out = np.bincount(src, minlength=N).astype(np.float32)
    deg_in = np.bincount(dst, minlength=N).astype(np.float32)

    order = np.lexsort((src, dst))
    s_srt, d_srt = src[order], dst[order]
    core_of = d_srt // NSH
    win_of = (d_srt - core_of * NSH) // W

    is_hi = s_srt > LO_MAX
    nlo = np.zeros((M, NW), np.int64)
    nhi = np.zeros((M, NW), np.int64)
    np.add.at(nlo, (core_of[~is_hi], win_of[~is_hi]), 1)
    np.add.at(nhi, (core_of[is_hi], win_of[is_hi]), 1)

    BL = np.maximum(1, -(-nlo.max(axis=0) // 128))
    BH = -(-nhi.max(axis=0) // 128)
    BT = BL + BH
    NB = int(BT.sum())

    idx16 = np.zeros((M, NB * 128), np.int16)
    dstloc = np.zeros((M, 128, NB), np.float32)
    key_half = is_hi.astype(np.int64)
    order2 = np.lexsort((s_srt, key_half, win_of, core_of))
    s2, d2 = s_srt[order2], d_srt[order2]
    c2, w2_, h2 = core_of[order2], win_of[order2], key_half[order2]

    woff = np.concatenate([[0], np.cumsum(BT)])
    counts = np.zeros((M, NW, 2), np.int64)
    np.add.at(counts, (c2, w2_, h2), 1)
    dstloc[:] = -1.0  # pad slots map to no dst slot (S one-hot row is zero)
    ptr = 0
    gch = GATHER_CHUNK if GATHER_CHUNK > 0 else 10**9
    for k in range(M):
        for w in range(NW):
            cl, ch = counts[k, w, 0], counts[k, w, 1]
            base_blk = woff[w]
            seg = slice(ptr, ptr + cl)
            pos = np.arange(cl)
            idx16[k, base_blk * 128 + pos] = (s2[seg] + 1).astype(np.int16)
            dstloc[k, pos % 128, base_blk + pos // 128] = (d2[seg] - k * NSH - w * W).astype(np.float32)
            ptr += cl
            pad_real = True  # -1 skip-pads crash the gather ucode on HW
            if cl < BL[w] * 128:
                pad = np.arange(cl, BL[w] * 128)
                idx16[k, base_blk * 128 + pad] = 0 if pad_real else -1
            base_blk_h = woff[w] + BL[w]
            seg = slice(ptr, ptr + ch)
            pos = np.arange(ch)
            idx16[k, base_blk_h * 128 + pos] = (s2[seg] - LO_MAX - 1).astype(np.int16)
            dstloc[k, pos % 128, base_blk_h + pos // 128] = (d2[seg] - k * NSH - w * W).astype(np.float32)
            ptr += ch
            if ch < BH[w] * 128:
                pad = np.arange(ch, BH[w] * 128)
                idx16[k, base_blk_h * 128 + pad] = HI_PAD_IDX if pad_real else -1
            # each of the 16 wrapped index lanes maps to one SDMA engine and
            # the DMA completion semaphore needs every engine to emit: keep
            # >=16 real (non-negative) indices at the head of every call.
            for sec_lo, sec_nb, fill in ((base_blk, int(BL[w]), 0),
                                         (base_blk_h, int(BH[w]), HI_PAD_IDX)):
                for s in range(0, sec_nb, gch):
                    e = min(s + gch, sec_nb)
                    sl = idx16[k, (sec_lo + s) * 128:(sec_lo + e) * 128]
                    if (sl[:16] < 0).any():
                        sl[:16][sl[:16] < 0] = fill
    assert ptr == E

    idx16_wrapped = np.tile(idx16.reshape(M, -1, 16).transpose(0, 2, 1), (1, 8, 1)).copy()

    def shard_deg(deg):
        out = np.zeros((M, 128, NW), np.float32)
        for k in range(M):
            d = deg[k * NSH:(k + 1) * NSH]
            d = np.concatenate([d, np.zeros(NW * W - NSH, np.float32)])
            out[k] = d.reshape(NW, W).T
        return out

    return dict(BL=BL, BH=BH, BT=BT, NB=NB, idx16_wrapped=idx16_wrapped, dstloc=dstloc,
                deg_in_sh=shard_deg(deg_in), deg_out_sh=shard_deg(deg_out), woff=woff)


def host_phase_a_inputs(feat, w1, w2, pp, dm):
    M, NSH, DIN, DOUT, RANK = dm["M"], dm["NSH"], dm["DIN"], dm["DOUT"], dm["RANK"]
    featT = np.ascontiguousarray(np.asarray(feat).T)
    Wcat = np.ascontiguousarray(np.concatenate([w1, w2[:DIN]], axis=1), dtype=np.float32)
    brow = np.zeros((1, DOUT + RANK), np.float32)
    brow[0, DOUT:] = w2[DIN]
    return [dict(featT=np.ascontiguousarray(featT[:, k * NSH:(k + 1) * NSH]),
                 Wcat=Wcat, brow=brow, deg_out_sh=pp["deg_out_sh"][k])
            for k in range(M)]


def assemble_T(shards, dm):
    width = shards[0].shape[1]
    T = np.zeros((dm["T_ROWS"], width), shards[0].dtype)
    for k in range(dm["M"]):
        T[1 + k * dm["NSH"]:1 + (k + 1) * dm["NSH"]] = shards[k][:dm["NSH"]]
    return T


def host_phase_b_inputs(T, pp, vmat, dm):
    return [dict(T=T, idx16w=pp["idx16_wrapped"][k], dstloc=pp["dstloc"][k],
                 deg_in_sh=pp["deg_in_sh"][k], vmat=np.ascontiguousarray(vmat, np.float32))
            for k in range(dm["M"])]


# ---------------- device kernels ----------------

@with_exitstack
def build_phase_a(ctx, tc, outs, ins, cfg):
    nc = tc.nc
    NSH, NW = cfg["NSH"], cfg["NW"]
    DIN = cfg["DIN"]
    DO, RK = cfg["DOUT"], cfg["RANK"]
    DT = DO + RK
    KC = DIN // 128

    cpool = ctx.enter_context(tc.tile_pool(name="const", bufs=1))
    wpool = ctx.enter_context(tc.tile_pool(name="work", bufs=4))
    ppool = ctx.enter_context(tc.tile_pool(name="psum", bufs=4, space="PSUM"))

    featT = cpool.tile([128, KC, NSH], BF16)
    nc.gpsimd.dma_start(out=featT[:],
                        in_=ins["featT"].rearrange("(c p) n -> p c n", p=128))
    Wsb32 = cpool.tile([128, KC, DT], F32)
    nc.sync.dma_start(out=Wsb32[:],
                      in_=ins["Wcat"].rearrange("(c p) n -> p c n", p=128))
    Wsb = cpool.tile([128, KC, DT], BF16)
    nc.vector.tensor_copy(out=Wsb[:], in_=Wsb32[:])
    bsb = cpool.tile([1, DT], F32)
    nc.sync.dma_start(out=bsb[:], in_=ins["brow"][:])
    ones = cpool.tile([1, 128], F32)
    nc.vector.memset(ones[:], 1.0)
    # replicate bias across partitions once: brep = ones.T @ bsb
    bps = ppool.tile([128, DT], F32, tag="bps")
    nc.tensor.matmul(out=bps[:], lhsT=ones[:], rhs=bsb[:], start=True, stop=True)
    brep = cpool.tile([128, DT], F32)
    nc.vector.tensor_copy(out=brep[:], in_=bps[:])
    deg = cpool.tile([128, NW], F32)
    nc.sync.dma_start(out=deg[:], in_=ins["deg_out_sh"][:])
    s_all = cpool.tile([128, NW], F32)
    nc.vector.tensor_scalar(out=s_all[:], in0=deg[:], scalar1=1.0, scalar2=None, op0=ALU.max)
    nc.scalar.activation(out=s_all[:], in_=s_all[:], func=AF.Sqrt)
    nc.vector.reciprocal(out=s_all[:], in_=s_all[:])

    # pass 1: all window matmuls + per-node scale into one staging buffer
    stall = cpool.tile([128, NW, DT], F32)
    for t in range(NW):
        nt = min(128, NSH - t * 128)
        ps = ppool.tile([128, DT], F32, tag="ps")
        for c in range(KC):
            nc.tensor.matmul(out=ps[:nt, :], lhsT=featT[:, c, t * 128:t * 128 + nt],
                             rhs=Wsb[:, c, :], start=(c == 0), stop=(c == KC - 1))
        if nt < 128:
            nc.vector.memset(stall[:, t, :], 0.0)
        nc.scalar.activation(out=stall[:nt, t, :], in_=ps[:nt, :], func=AF.Copy,
                             scale=s_all[:nt, t:t + 1])
    # pass 2: batched elementwise over all windows (one table load per func)
    nc.vector.tensor_tensor(out=stall[:], in0=stall[:],
                            in1=brep[:].unsqueeze(1).to_broadcast([128, NW, DT]),
                            op=ALU.add)
    tnh = cpool.tile([128, NW, RK], F32)
    nc.scalar.activation(out=tnh[:], in_=stall[:, :, DO:DT], func=AF.Tanh)
    sto = cpool.tile([128, NW, TROW], BF16)
    # neg indicator from tanh sign, then |tanh| -> ln -> clamp -> |ln|
    nc.vector.tensor_scalar(out=sto[:, :, DT:TROW], in0=tnh[:],
                            scalar1=0.0, scalar2=None, op0=ALU.is_lt)
    nc.vector.tensor_scalar(out=tnh[:].bitcast(I32), in0=tnh[:].bitcast(I32),
                            scalar1=MASK_ABS, scalar2=None, op0=ALU.bitwise_and)
    nc.scalar.activation(out=tnh[:], in_=tnh[:], func=AF.Ln)
    nc.vector.tensor_scalar(out=tnh[:], in0=tnh[:], scalar1=-1e-7, scalar2=None, op0=ALU.min)
    nc.vector.tensor_scalar(out=stall[:, :, DO:DT].bitcast(I32), in0=tnh[:].bitcast(I32),
                            scalar1=MASK_ABS, scalar2=None, op0=ALU.bitwise_and)
    nc.vector.tensor_copy(out=sto[:, :, 0:DT], in_=stall[:])
    nc.sync.dma_start(out=outs["Tsh"].rearrange("(w p) f -> p w f", p=128), in_=sto[:])


@with_exitstack
def build_phase_b(ctx, tc, outs, ins, cfg):
    nc = tc.nc
    NW = cfg["NW"]
    DO, RK = cfg["DOUT"], cfg["RANK"]
    DT = DO + RK
    BL, BH = cfg["BL"], cfg["BH"]
    woff = cfg["woff"]
    NB = cfg["NB"]
    HI_BASE = cfg["HI_BASE"]
    T_ROWS = cfg["T_ROWS"]
    GROUP = cfg.get("GROUP", 12)
    NBMAX = int(max(BL[w] + BH[w] for w in range(NW)))

    cpool = ctx.enter_context(tc.tile_pool(name="const", bufs=1))
    gpool = ctx.enter_context(tc.tile_pool(name="g", bufs=3))
    spool = ctx.enter_context(tc.tile_pool(name="s", bufs=2))
    hpool = ctx.enter_context(tc.tile_pool(name="h", bufs=2))
    epool = ctx.enter_context(tc.tile_pool(name="e", bufs=2))
    rpool = ctx.enter_context(tc.tile_pool(name="r", bufs=2))
    accp = ctx.enter_context(tc.tile_pool(name="acc", bufs=2, space="PSUM"))
    tpp = ctx.enter_context(tc.tile_pool(name="tp", bufs=2, space="PSUM"))
    vpp = ctx.enter_context(tc.tile_pool(name="vp", bufs=2, space="PSUM"))

    idxs = cpool.tile([128, NB * 8], I16)
    nc.sync.dma_start(out=idxs[:], in_=ins["idx16w"][:])
    dl = cpool.tile([128, NB], F32)
    nc.sync.dma_start(out=dl[:], in_=ins["dstloc"][:])
    deg = cpool.tile([128, NW], F32)
    nc.sync.dma_start(out=deg[:], in_=ins["deg_in_sh"][:])
    vm = cpool.tile([64, 128], F32)
    nc.sync.dma_start(out=vm[:RK, :DO], in_=ins["vmat"][:])
    ident = cpool.tile([128, 128], F32)
    make_identity(nc, ident[:])
    iota_i = cpool.tile([128, 128], I32)
    nc.gpsimd.iota(iota_i[:], pattern=[[1, 128]], base=0, channel_multiplier=0)
    iota_f = cpool.tile([128, 128], F32)
    nc.vector.tensor_copy(out=iota_f[:], in_=iota_i[:])
    mask = cpool.tile([128, NW], F32)
    nc.vector.tensor_scalar(out=mask[:], in0=deg[:], scalar1=0.0, scalar2=None, op0=ALU.is_gt)
    inv = cpool.tile([128, NW], F32)
    nc.vector.tensor_scalar(out=inv[:], in0=deg[:], scalar1=1.0, scalar2=None, op0=ALU.max)
    nc.scalar.activation(out=inv[:], in_=inv[:], func=AF.Sqrt)
    nc.vector.reciprocal(out=inv[:], in_=inv[:])

    TLO = ins["T"][0:HI_BASE, :]
    THI = ins["T"][HI_BASE:T_ROWS, :] if any(BH[w] > 0 for w in range(NW)) else None

    out_r = outs["out_sh"].rearrange("(w p) f -> p w f", p=128)

    qn = [0]

    w = 0
    while w < NW:
        ws = list(range(w, min(w + GROUP, NW)))
        nwg = len(ws)
        g0 = ws[0]
        H = hpool.tile([128, nwg, 256], F32, tag="H")
        for j, wi in enumerate(ws):
            nb = int(BL[wi] + BH[wi])
            b0 = int(woff[wi])
            G = gpool.tile([128, NBMAX, TROW], BF16, tag="G")

            def _gather(tbl, blk_lo, blk_hi):
                step = GATHER_CHUNK if GATHER_CHUNK > 0 else (blk_hi - blk_lo)
                for s in range(blk_lo, blk_hi, step):
                    e = min(s + step, blk_hi)
                    ni = (e - s) * 128
                    nc.gpsimd.dma_gather(
                        out_ap=G[:, s:e, :], in_ap=tbl,
                        idxs_ap=idxs[:, (b0 + s) * 8:(b0 + e) * 8],
                        num_idxs=ni, num_idxs_reg=ni, elem_size=TROW,
                        single_packet=SINGLE_PACKET,
                        queue_num=qn[0] % NQUEUE)
                    qn[0] += 1

            _gather(TLO, 0, int(BL[wi]))
            if BH[wi] > 0:
                _gather(THI, int(BL[wi]), nb)
            S = spool.tile([128, NBMAX, 128], BF16, tag="S")
            nc.vector.tensor_tensor(
                out=S[:, :nb, :],
                in0=dl[:, b0:b0 + nb].unsqueeze(2).to_broadcast([128, nb, 128]),
                in1=iota_f[:].unsqueeze(1).to_broadcast([128, nb, 128]),
                op=ALU.is_equal)
            ps1 = accp.tile([128, 256], F32, tag="acc1")
            for b in range(nb):
                nc.tensor.matmul(out=ps1[:], lhsT=S[:, b, :], rhs=G[:, b, :],
                                 start=(b == 0), stop=(b == nb - 1))
            if j % 2 == 0:
                nc.vector.tensor_copy(out=H[:, j, :], in_=ps1[:])
            else:
                nc.scalar.activation(out=H[:, j, :], in_=ps1[:], func=AF.Copy)
        # epilogue for the group
        pi = epool.tile([128, GROUP, RK], I32, tag="pi")
        nc.vector.tensor_copy(out=pi[:, :nwg, :], in_=H[:, :, DO + RK:256])
        nc.vector.tensor_scalar(out=pi[:, :nwg, :], in0=pi[:, :nwg, :], scalar1=1,
                                scalar2=None, op0=ALU.bitwise_and)
        sg = epool.tile([128, GROUP, RK], F32, tag="sg")
        nc.vector.tensor_copy(out=sg[:, :nwg, :], in_=pi[:, :nwg, :])
        nc.vector.tensor_scalar(out=sg[:, :nwg, :], in0=sg[:, :nwg, :], scalar1=-2.0,
                                scalar2=1.0, op0=ALU.mult, op1=ALU.add)
        ex = epool.tile([128, GROUP, RK], F32, tag="ex")
        nc.scalar.activation(out=ex[:, :nwg, :], in_=H[:, :, DO:DO + RK], func=AF.Exp, scale=-1.0)
        nc.vector.tensor_tensor(out=ex[:, :nwg, :], in0=ex[:, :nwg, :], in1=sg[:, :nwg, :], op=ALU.mult)
        nc.vector.tensor_tensor(
            out=ex[:, :nwg, :], in0=ex[:, :nwg, :],
            in1=mask[:, g0:g0 + nwg].unsqueeze(2).to_broadcast([128, nwg, RK]),
            op=ALU.mult)
        RST = rpool.tile([128, GROUP, DO], F32, tag="RST")
        for j, wi in enumerate(ws):
            tp = tpp.tile([64, 128], F32, tag="tp")
            nc.tensor.transpose(out=tp[:RK, :], in_=ex[:, j, :], identity=ident[:])
            hpT = epool.tile([64, 128], F32, tag="hpT")
            nc.vector.tensor_copy(out=hpT[:RK, :], in_=tp[:RK, :])
            vp = vpp.tile([128, DO], F32, tag="vpp")
            nc.tensor.matmul(out=vp[:], lhsT=hpT[:RK, :], rhs=vm[:RK, :DO], start=True, stop=True)
            nc.vector.tensor_tensor(out=RST[:, j, :], in0=H[:, j, 0:DO], in1=vp[:], op=ALU.add)
        nc.vector.tensor_tensor(
            out=RST[:, :nwg, :], in0=RST[:, :nwg, :],
            in1=inv[:, g0:g0 + nwg].unsqueeze(2).to_broadcast([128, nwg, DO]),
            op=ALU.mult)
        nc.sync.dma_start(out=out_r[:, g0:g0 + nwg, :], in_=RST[:, :nwg, :])
        w += GROUP


# ---------------- SPMD drivers ----------------

def _new_nc(n_cores):
    return bacc.Bacc("TRN2", target_bir_lowering=False, debug=False,
                     enable_asserts=False, num_devices=n_cores)


def _build_a(dm):
    nc = _new_nc(dm["M"])
    DIN, NSH, NW = dm["DIN"], dm["NSH"], dm["NW"]
    DT = dm["DOUT"] + dm["RANK"]
    ins = dict(
        featT=nc.dram_tensor("featT", [DIN, NSH], F32, kind="ExternalInput").ap(),
        Wcat=nc.dram_tensor("Wcat", [DIN, DT], F32, kind="ExternalInput").ap(),
        brow=nc.dram_tensor("brow", [1, DT], F32, kind="ExternalInput").ap(),
        deg_out_sh=nc.dram_tensor("deg_out_sh", [128, NW], F32, kind="ExternalInput").ap(),
    )
    outs = dict(Tsh=nc.dram_tensor("Tsh", [NW * 128, TROW], BF16, kind="ExternalOutput").ap())
    cfg = dict(NSH=NSH, NW=NW, DIN=DIN, DOUT=dm["DOUT"], RANK=dm["RANK"])
    with tile.TileContext(nc) as tc:
        build_phase_a(tc, outs, ins, cfg)
    nc.compile()
    return nc


def _build_b(dm, pp, group=12):
    nc = _new_nc(dm["M"])
    NW, NB = dm["NW"], pp["NB"]
    ins = dict(
        T=nc.dram_tensor("T", [dm["T_ROWS"], TROW], BF16, kind="ExternalInput").ap(),
        idx16w=nc.dram_tensor("idx16w", [128, NB * 8], I16, kind="ExternalInput").ap(),
        dstloc=nc.dram_tensor("dstloc", [128, NB], F32, kind="ExternalInput").ap(),
        deg_in_sh=nc.dram_tensor("deg_in_sh", [128, NW], F32, kind="ExternalInput").ap(),
        vmat=nc.dram_tensor("vmat", [dm["RANK"], dm["DOUT"]], F32, kind="ExternalInput").ap(),
    )
    outs = dict(out_sh=nc.dram_tensor("out_sh", [NW * 128, dm["DOUT"]], F32,
                                      kind="ExternalOutput").ap())
    cfg = dict(NW=NW, DOUT=dm["DOUT"], RANK=dm["RANK"], BL=pp["BL"], BH=pp["BH"],
               woff=pp["woff"], NB=NB, HI_BASE=dm["HI_BASE"], T_ROWS=dm["T_ROWS"],
               GROUP=group)
    with tile.TileContext(nc) as tc:
        build_phase_b(tc, outs, ins, cfg)
    nc.compile()
    return nc


def run_all(feat, w1, w2, v, src, dst, trace=False, tmpdir_a=None, tmpdir_b=None):
    """Returns (output, info dict with per-launch BassKernelResults)."""
    dm = make_dims(N=feat.shape[0], E=src.shape[0], DIN=feat.shape[1],
                   DOUT=w1.shape[1], RANK=v.shape[0])
    pp = preprocess(src, dst, dm)
    M, NSH = dm["M"], dm["NSH"]

    ains = host_phase_a_inputs(feat, w1, w2, pp, dm)
    nc_a = _build_a(dm)
    ra = bass_utils.run_bass_kernel_spmd(nc_a, ains, list(range(M)), trace=trace,
                                         tmpdir=tmpdir_a)
    shards = [ra.results[k]["Tsh"] for k in range(M)]
    T = assemble_T(shards, dm)

    bins = host_phase_b_inputs(T, pp, v, dm)
    nc_b = _build_b(dm, pp)
    rb = bass_utils.run_bass_kernel_spmd(nc_b, bins, list(range(M)), trace=trace,
                                         tmpdir=tmpdir_b)
    out = np.concatenate([rb.results[k]["out_sh"][:NSH] for k in range(M)], axis=0)
    return out.astype(np.float32), dict(ra=ra, rb=rb, dm=dm, pp=pp)


def kernel(feat, w1, w2, v, src, dst):
    feat = np.asarray(feat, np.float32)
    w1 = np.asarray(w1, np.float32)
    w2 = np.asarray(w2, np.float32)
    v = np.asarray(v, np.float32)
    src = np.asarray(src)
    dst = np.asarray(dst)
    out, _ = run_all(feat, w1, w2, v, src, dst, trace=False)
    return out


# revision 20
# speedup vs baseline: 1.1725x; 1.0328x over previous
"""Trainium2 Bass kernel for nn_DGLGraphConv (graph conv with sum- and product-reduce).

Strategy (8 NeuronCores, SPMD, two launches):
  Launch A (node-sharded, 6250 nodes/core): per-node table, bf16, 256 cols:
      T[n] = [ (feat@w1)*s_out (128) | |log|tanh|| (64) | neg indicator (64) ]
  Host: concat T shards (pure relayout; row 0 / tail rows are zeros used as gather padding).
  Launch B (dst-sharded by edge partitioning): per core, 49 windows x 128 dst slots.
      Edges sorted by (dst window, src half, src), padded to 128-edge blocks with
      block counts maxed over cores so all cores run one program. Per block:
      dma_gather 128 rows of T (512B each), one-hot S matmul accumulates
      [sum(fs) | sum|ln| | count(neg)] per dst slot in PSUM; epilogue computes
      sign via parity, exp, mask, @v, and the in-degree scaling.

Host does integer index prep (bincount/sort/pad) and layout-only transforms;
all floating-point math runs on device.
"""
import sys
from contextlib import ExitStack

import numpy as np

for _p in ("/opt/trn_rl_repo",):
    if _p not in sys.path:
        sys.path.insert(0, _p)

import concourse.bass as bass
import concourse.mybir as mybir
import concourse.tile as tile
from concourse import bacc, bass_utils
from concourse._compat import with_exitstack
from concourse.masks import make_identity

F32 = mybir.dt.float32
I32 = mybir.dt.int32
I16 = mybir.dt.int16
BF16 = mybir.dt.bfloat16
AF = mybir.ActivationFunctionType
ALU = mybir.AluOpType

MASK_ABS = 0x7FFFFFFF
MASK_SGN = -0x80000000

# T row layout (bf16, 256 cols = 512B):
#   [fs(128) | absl(64) | neg(64)]
TROW = 256
import os as _os
# Max 128-edge blocks per dma_gather call. Full-window calls (~2200 emitted
# descriptors) die on HW (descriptor-ring overflow); pad slots are emitted as
# trailing -1 (skipped by the ucode), keeping per-call emission ~<=1450.
GATHER_CHUNK = int(_os.environ.get("GCH", "8"))
SINGLE_PACKET = _os.environ.get("SPKT", "0") == "1"
NQUEUE = int(_os.environ.get("NQ", "1"))  # rotate gather calls over SWDGE queues
# windows whose G buffer may be uninitialized SBUF (NaN patterns): gather real
# zero rows for padding there instead of skipping, so 0*NaN never reaches PSUM.
PAD_REAL_WINDOWS = 3


# ---------------- host-side prep (integer/layout only) ----------------

def make_dims(N=50000, E=800000, DIN=256, DOUT=128, RANK=64, M=8, LO_MAX=32766):
    LO_MAX = min(LO_MAX, N - 1)
    NSH = N // M
    W = 128
    NW = (NSH + W - 1) // W
    T_ROWS = N + 4
    HI_BASE = LO_MAX + 2
    NHI = T_ROWS - HI_BASE
    return dict(N=N, E=E, DIN=DIN, DOUT=DOUT, RANK=RANK, M=M, NSH=NSH, W=W, NW=NW,
                LO_MAX=LO_MAX, T_ROWS=T_ROWS, HI_BASE=HI_BASE, NHI=NHI,
                HI_PAD_IDX=NHI - 2)


def preprocess(src, dst, dm):
    N, E, M, NSH, W, NW = dm["N"], dm["E"], dm["M"], dm["NSH"], dm["W"], dm["NW"]
    LO_MAX, HI_PAD_IDX = dm["LO_MAX"], dm["HI_PAD_IDX"]
    src = np.asarray(src).astype(np.int64)
    dst = np.asarray(dst).astype(np.int64)
    deg_out = np.bincount(src, minlength=N).astype(np.float32)
    deg_in = np.bincount(dst, minlength=N).astype(np.float32)

    order = np.lexsort((src, dst))
    s_srt, d_srt = src[order], dst[order]
    core_of = d_srt // NSH
    win_of = (d_srt - core_of * NSH) // W

    is_hi = s_srt > LO_MAX
    nlo = np.zeros((M, NW), np.int64)
    nhi = np.zeros((M, NW), np.int64)
    np.add.at(nlo, (core_of[~is_hi], win_of[~is_hi]), 1)
    np.add.at(nhi, (core_of[is_hi], win_of[is_hi]), 1)

    BL = np.maximum(1, -(-nlo.max(axis=0) // 128))
    BH = -(-nhi.max(axis=0) // 128)
    BT = BL + BH
    NB = int(BT.sum())

    idx16 = np.zeros((M, NB * 128), np.int16)
    dstloc = np.zeros((M, 128, NB), np.float32)
    key_half = is_hi.astype(np.int64)
    order2 = np.lexsort((s_srt, key_half, win_of, core_of))
    s2, d2 = s_srt[order2], d_srt[order2]
    c2, w2_, h2 = core_of[order2], win_of[order2], key_half[order2]

    woff = np.concatenate([[0], np.cumsum(BT)])
    counts = np.zeros((M, NW, 2), np.int64)
    np.add.at(counts, (c2, w2_, h2), 1)
    dstloc[:] = -1.0  # pad slots map to no dst slot (S one-hot row is zero)
    ptr = 0
    gch = GATHER_CHUNK if GATHER_CHUNK > 0 else 10**9
    for k in range(M):
        for w in range(NW):
            cl, ch = counts[k, w, 0], counts[k, w, 1]
            base_blk = woff[w]
            seg = slice(ptr, ptr + cl)
            pos = np.arange(cl)
            idx16[k, base_blk * 128 + pos] = (s2[seg] + 1).astype(np.int16)
            dstloc[k, pos % 128, base_blk + pos // 128] = (d2[seg] - k * NSH - w * W).astype(np.float32)
            ptr += cl
            pad_real = True  # -1 skip-pads crash the gather ucode on HW
            if cl < BL[w] * 128:
                pad = np.arange(cl, BL[w] * 128)
                idx16[k, base_blk * 128 + pad] = 0 if pad_real else -1
            base_blk_h = woff[w] + BL[w]
            seg = slice(ptr, ptr + ch)
            pos = np.arange(ch)
            idx16[k, base_blk_h * 128 + pos] = (s2[seg] - LO_MAX - 1).astype(np.int16)
            dstloc[k, pos % 128, base_blk_h + pos // 128] = (d2[seg] - k * NSH - w * W).astype(np.float32)
            ptr += ch
            if ch < BH[w] * 128:
                pad = np.arange(ch, BH[w] * 128)
                idx16[k, base_blk_h * 128 + pad] = HI_PAD_IDX if pad_real else -1
            # each of the 16 wrapped index lanes maps to one SDMA engine and
            # the DMA completion semaphore needs every engine to emit: keep
            # >=16 real (non-negative) indices at the head of every call.
            for sec_lo, sec_nb, fill in ((base_blk, int(BL[w]), 0),
                                         (base_blk_h, int(BH[w]), HI_PAD_IDX)):
                for s in range(0, sec_nb, gch):
                    e = min(s + gch, sec_nb)
                    sl = idx16[k, (sec_lo + s) * 128:(sec_lo + e) * 128]
                    if (sl[:16] < 0).any():
                        sl[:16][sl[:16] < 0] = fill
    assert ptr == E

    idx16_wrapped = np.tile(idx16.reshape(M, -1, 16).transpose(0, 2, 1), (1, 8, 1)).copy()

    def shard_deg(deg):
        out = np.zeros((M, 128, NW), np.float32)
        for k in range(M):
            d = deg[k * NSH:(k + 1) * NSH]
            d = np.concatenate([d, np.zeros(NW * W - NSH, np.float32)])
            out[k] = d.reshape(NW, W).T
        return out

    return dict(BL=BL, BH=BH, BT=BT, NB=NB, idx16_wrapped=idx16_wrapped, dstloc=dstloc,
                deg_in_sh=shard_deg(deg_in), deg_out_sh=shard_deg(deg_out), woff=woff,
                ni_lo=counts[:, :, 0].max(axis=0), ni_hi=counts[:, :, 1].max(axis=0))


def host_phase_a_inputs(feat, w1, w2, pp, dm):
    M, NSH, DIN, DOUT, RANK = dm["M"], dm["NSH"], dm["DIN"], dm["DOUT"], dm["RANK"]
    featT = np.ascontiguousarray(np.asarray(feat).T)
    Wcat = np.ascontiguousarray(np.concatenate([w1, w2[:DIN]], axis=1), dtype=np.float32)
    brow = np.zeros((1, DOUT + RANK), np.float32)
    brow[0, DOUT:] = w2[DIN]
    return [dict(featT=np.ascontiguousarray(featT[:, k * NSH:(k + 1) * NSH]),
                 Wcat=Wcat, brow=brow, deg_out_sh=pp["deg_out_sh"][k])
            for k in range(M)]


def assemble_T(shards, dm):
    width = shards[0].shape[1]
    T = np.zeros((dm["T_ROWS"], width), shards[0].dtype)
    for k in range(dm["M"]):
        T[1 + k * dm["NSH"]:1 + (k + 1) * dm["NSH"]] = shards[k][:dm["NSH"]]
    return T


def host_phase_b_inputs(T, pp, vmat, dm):
    return [dict(T=T, idx16w=pp["idx16_wrapped"][k], dstloc=pp["dstloc"][k],
                 deg_in_sh=pp["deg_in_sh"][k], vmat=np.ascontiguousarray(vmat, np.float32))
            for k in range(dm["M"])]


# ---------------- device kernels ----------------

@with_exitstack
def build_phase_a(ctx, tc, outs, ins, cfg):
    nc = tc.nc
    NSH, NW = cfg["NSH"], cfg["NW"]
    DIN = cfg["DIN"]
    DO, RK = cfg["DOUT"], cfg["RANK"]
    DT = DO + RK
    KC = DIN // 128

    cpool = ctx.enter_context(tc.tile_pool(name="const", bufs=1))
    wpool = ctx.enter_context(tc.tile_pool(name="work", bufs=4))
    ppool = ctx.enter_context(tc.tile_pool(name="psum", bufs=4, space="PSUM"))

    featT = cpool.tile([128, KC, NSH], BF16)
    nc.gpsimd.dma_start(out=featT[:],
                        in_=ins["featT"].rearrange("(c p) n -> p c n", p=128))
    Wsb32 = cpool.tile([128, KC, DT], F32)
    nc.sync.dma_start(out=Wsb32[:],
                      in_=ins["Wcat"].rearrange("(c p) n -> p c n", p=128))
    Wsb = cpool.tile([128, KC, DT], BF16)
    nc.vector.tensor_copy(out=Wsb[:], in_=Wsb32[:])
    bsb = cpool.tile([1, DT], F32)
    nc.sync.dma_start(out=bsb[:], in_=ins["brow"][:])
    ones = cpool.tile([1, 128], F32)
    nc.vector.memset(ones[:], 1.0)
    # replicate bias across partitions once: brep = ones.T @ bsb
    bps = ppool.tile([128, DT], F32, tag="bps")
    nc.tensor.matmul(out=bps[:], lhsT=ones[:], rhs=bsb[:], start=True, stop=True)
    brep = cpool.tile([128, DT], F32)
    nc.vector.tensor_copy(out=brep[:], in_=bps[:])
    deg = cpool.tile([128, NW], F32)
    nc.sync.dma_start(out=deg[:], in_=ins["deg_out_sh"][:])
    s_all = cpool.tile([128, NW], F32)
    nc.vector.tensor_scalar(out=s_all[:], in0=deg[:], scalar1=1.0, scalar2=None, op0=ALU.max)
    nc.scalar.activation(out=s_all[:], in_=s_all[:], func=AF.Sqrt)
    nc.vector.reciprocal(out=s_all[:], in_=s_all[:])

    # pass 1: all window matmuls + per-node scale into one staging buffer
    stall = cpool.tile([128, NW, DT], F32)
    for t in range(NW):
        nt = min(128, NSH - t * 128)
        ps = ppool.tile([128, DT], F32, tag="ps")
        for c in range(KC):
            nc.tensor.matmul(out=ps[:nt, :], lhsT=featT[:, c, t * 128:t * 128 + nt],
                             rhs=Wsb[:, c, :], start=(c == 0), stop=(c == KC - 1))
        if nt < 128:
            nc.vector.memset(stall[:, t, :], 0.0)
        nc.scalar.activation(out=stall[:nt, t, :], in_=ps[:nt, :], func=AF.Copy,
                             scale=s_all[:nt, t:t + 1])
    # pass 2: batched elementwise over all windows (one table load per func)
    nc.vector.tensor_tensor(out=stall[:], in0=stall[:],
                            in1=brep[:].unsqueeze(1).to_broadcast([128, NW, DT]),
                            op=ALU.add)
    tnh = cpool.tile([128, NW, RK], F32)
    nc.scalar.activation(out=tnh[:], in_=stall[:, :, DO:DT], func=AF.Tanh)
    sto = cpool.tile([128, NW, TROW], BF16)
    # neg indicator from tanh sign, then |tanh| -> ln -> clamp -> |ln|
    nc.vector.tensor_scalar(out=sto[:, :, DT:TROW], in0=tnh[:],
                            scalar1=0.0, scalar2=None, op0=ALU.is_lt)
    nc.vector.tensor_scalar(out=tnh[:].bitcast(I32), in0=tnh[:].bitcast(I32),
                            scalar1=MASK_ABS, scalar2=None, op0=ALU.bitwise_and)
    nc.scalar.activation(out=tnh[:], in_=tnh[:], func=AF.Ln)
    nc.vector.tensor_scalar(out=tnh[:], in0=tnh[:], scalar1=-1e-7, scalar2=None, op0=ALU.min)
    nc.vector.tensor_scalar(out=stall[:, :, DO:DT].bitcast(I32), in0=tnh[:].bitcast(I32),
                            scalar1=MASK_ABS, scalar2=None, op0=ALU.bitwise_and)
    nc.vector.tensor_copy(out=sto[:, :, 0:DT], in_=stall[:])
    nc.sync.dma_start(out=outs["Tsh"].rearrange("(w p) f -> p w f", p=128), in_=sto[:])


@with_exitstack
def build_phase_b(ctx, tc, outs, ins, cfg):
    nc = tc.nc
    NW = cfg["NW"]
    DO, RK = cfg["DOUT"], cfg["RANK"]
    DT = DO + RK
    BL, BH = cfg["BL"], cfg["BH"]
    woff = cfg["woff"]
    NB = cfg["NB"]
    HI_BASE = cfg["HI_BASE"]
    T_ROWS = cfg["T_ROWS"]
    GROUP = cfg.get("GROUP", 12)
    NBMAX = int(max(BL[w] + BH[w] for w in range(NW)))

    cpool = ctx.enter_context(tc.tile_pool(name="const", bufs=1))
    spool = ctx.enter_context(tc.tile_pool(name="s", bufs=2))
    hpool = ctx.enter_context(tc.tile_pool(name="h", bufs=2))
    epool = ctx.enter_context(tc.tile_pool(name="e", bufs=2))
    rpool = ctx.enter_context(tc.tile_pool(name="r", bufs=2))
    accp = ctx.enter_context(tc.tile_pool(name="acc", bufs=2, space="PSUM"))
    tpp = ctx.enter_context(tc.tile_pool(name="tp", bufs=2, space="PSUM"))
    vpp = ctx.enter_context(tc.tile_pool(name="vp", bufs=2, space="PSUM"))

    idxs = cpool.tile([128, NB * 8], I16)
    nc.sync.dma_start(out=idxs[:], in_=ins["idx16w"][:])
    dl = cpool.tile([128, NB], F32)
    nc.sync.dma_start(out=dl[:], in_=ins["dstloc"][:])
    deg = cpool.tile([128, NW], F32)
    nc.sync.dma_start(out=deg[:], in_=ins["deg_in_sh"][:])
    vm = cpool.tile([64, 128], F32)
    nc.sync.dma_start(out=vm[:RK, :DO], in_=ins["vmat"][:])
    ident = cpool.tile([128, 128], F32)
    make_identity(nc, ident[:])
    iota_i = cpool.tile([128, 128], I32)
    nc.gpsimd.iota(iota_i[:], pattern=[[1, 128]], base=0, channel_multiplier=0)
    iota_f = cpool.tile([128, 128], F32)
    nc.vector.tensor_copy(out=iota_f[:], in_=iota_i[:])
    mask = cpool.tile([128, NW], F32)
    nc.vector.tensor_scalar(out=mask[:], in0=deg[:], scalar1=0.0, scalar2=None, op0=ALU.is_gt)
    inv = cpool.tile([128, NW], F32)
    nc.vector.tensor_scalar(out=inv[:], in0=deg[:], scalar1=1.0, scalar2=None, op0=ALU.max)
    nc.scalar.activation(out=inv[:], in_=inv[:], func=AF.Sqrt)
    nc.vector.reciprocal(out=inv[:], in_=inv[:])

    TLO = ins["T"][0:HI_BASE, :]
    THI = ins["T"][HI_BASE:T_ROWS, :] if any(BH[w] > 0 for w in range(NW)) else None

    # manually rotated gather buffers, zeroed once: stale/unwritten slots must
    # stay finite (0 * NaN would poison the routing matmul's PSUM column).
    Gbuf = cpool.tile([128, 3, NBMAX, TROW], BF16)
    nc.vector.memset(Gbuf[:], 0.0)

    out_r = outs["out_sh"].rearrange("(w p) f -> p w f", p=128)

    qn = [0]

    w = 0
    while w < NW:
        ws = list(range(w, min(w + GROUP, NW)))
        nwg = len(ws)
        g0 = ws[0]
        H = hpool.tile([128, nwg, 256], F32, tag="H")
        for j, wi in enumerate(ws):
            nb = int(BL[wi] + BH[wi])
            b0 = int(woff[wi])
            G = Gbuf[:, wi % 3]

            def _gather(tbl, blk_lo, blk_hi, ni_total):
                # truncate to the real index count — stale tail slots map to
                # no dst slot (dstloc=-1 -> zero S row) and are finite.
                step = GATHER_CHUNK if GATHER_CHUNK > 0 else (blk_hi - blk_lo)
                done = 0
                for s in range(blk_lo, blk_hi, step):
                    e = min(s + step, blk_hi)
                    cap = (e - s) * 128
                    ni = min(max(ni_total - done, 0), cap)
                    done += cap
                    if ni == 0:
                        break
                    ni = max(ni, 16)
                    eb = s + (ni + 127) // 128
                    nc.gpsimd.dma_gather(
                        out_ap=G[:, s:eb, :], in_ap=tbl,
                        idxs_ap=idxs[:, (b0 + s) * 8:(b0 + eb) * 8],
                        num_idxs=ni, num_idxs_reg=ni, elem_size=TROW,
                        single_packet=SINGLE_PACKET,
                        queue_num=qn[0] % NQUEUE)
                    qn[0] += 1

            _gather(TLO, 0, int(BL[wi]), int(cfg["ni_lo"][wi]))
            if BH[wi] > 0:
                _gather(THI, int(BL[wi]), nb, int(cfg["ni_hi"][wi]))
            S = spool.tile([128, NBMAX, 128], BF16, tag="S")
            nc.vector.tensor_tensor(
                out=S[:, :nb, :],
                in0=dl[:, b0:b0 + nb].unsqueeze(2).to_broadcast([128, nb, 128]),
                in1=iota_f[:].unsqueeze(1).to_broadcast([128, nb, 128]),
                op=ALU.is_equal)
            ps1 = accp.tile([128, 256], F32, tag="acc1")
            for b in range(nb):
                nc.tensor.matmul(out=ps1[:], lhsT=S[:, b, :], rhs=G[:, b, :],
                                 start=(b == 0), stop=(b == nb - 1))
            if j % 2 == 0:
                nc.vector.tensor_copy(out=H[:, j, :], in_=ps1[:])
            else:
                nc.scalar.activation(out=H[:, j, :], in_=ps1[:], func=AF.Copy)
        # epilogue for the group
        pi = epool.tile([128, GROUP, RK], I32, tag="pi")
        nc.vector.tensor_copy(out=pi[:, :nwg, :], in_=H[:, :, DO + RK:256])
        nc.vector.tensor_scalar(out=pi[:, :nwg, :], in0=pi[:, :nwg, :], scalar1=1,
                                scalar2=None, op0=ALU.bitwise_and)
        sg = epool.tile([128, GROUP, RK], F32, tag="sg")
        nc.vector.tensor_copy(out=sg[:, :nwg, :], in_=pi[:, :nwg, :])
        nc.vector.tensor_scalar(out=sg[:, :nwg, :], in0=sg[:, :nwg, :], scalar1=-2.0,
                                scalar2=1.0, op0=ALU.mult, op1=ALU.add)
        ex = epool.tile([128, GROUP, RK], F32, tag="ex")
        nc.scalar.activation(out=ex[:, :nwg, :], in_=H[:, :, DO:DO + RK], func=AF.Exp, scale=-1.0)
        nc.vector.tensor_tensor(out=ex[:, :nwg, :], in0=ex[:, :nwg, :], in1=sg[:, :nwg, :], op=ALU.mult)
        nc.vector.tensor_tensor(
            out=ex[:, :nwg, :], in0=ex[:, :nwg, :],
            in1=mask[:, g0:g0 + nwg].unsqueeze(2).to_broadcast([128, nwg, RK]),
            op=ALU.mult)
        RST = rpool.tile([128, GROUP, DO], F32, tag="RST")
        for j, wi in enumerate(ws):
            tp = tpp.tile([64, 128], F32, tag="tp")
            nc.tensor.transpose(out=tp[:RK, :], in_=ex[:, j, :], identity=ident[:])
            hpT = epool.tile([64, 128], F32, tag="hpT")
            nc.vector.tensor_copy(out=hpT[:RK, :], in_=tp[:RK, :])
            vp = vpp.tile([128, DO], F32, tag="vpp")
            nc.tensor.matmul(out=vp[:], lhsT=hpT[:RK, :], rhs=vm[:RK, :DO], start=True, stop=True)
            nc.vector.tensor_tensor(out=RST[:, j, :], in0=H[:, j, 0:DO], in1=vp[:], op=ALU.add)
        nc.vector.tensor_tensor(
            out=RST[:, :nwg, :], in0=RST[:, :nwg, :],
            in1=inv[:, g0:g0 + nwg].unsqueeze(2).to_broadcast([128, nwg, DO]),
            op=ALU.mult)
        nc.sync.dma_start(out=out_r[:, g0:g0 + nwg, :], in_=RST[:, :nwg, :])
        w += GROUP


# ---------------- SPMD drivers ----------------

def _new_nc(n_cores):
    return bacc.Bacc("TRN2", target_bir_lowering=False, debug=False,
                     enable_asserts=False, num_devices=n_cores)


def _build_a(dm):
    nc = _new_nc(dm["M"])
    DIN, NSH, NW = dm["DIN"], dm["NSH"], dm["NW"]
    DT = dm["DOUT"] + dm["RANK"]
    ins = dict(
        featT=nc.dram_tensor("featT", [DIN, NSH], F32, kind="ExternalInput").ap(),
        Wcat=nc.dram_tensor("Wcat", [DIN, DT], F32, kind="ExternalInput").ap(),
        brow=nc.dram_tensor("brow", [1, DT], F32, kind="ExternalInput").ap(),
        deg_out_sh=nc.dram_tensor("deg_out_sh", [128, NW], F32, kind="ExternalInput").ap(),
    )
    outs = dict(Tsh=nc.dram_tensor("Tsh", [NW * 128, TROW], BF16, kind="ExternalOutput").ap())
    cfg = dict(NSH=NSH, NW=NW, DIN=DIN, DOUT=dm["DOUT"], RANK=dm["RANK"])
    with tile.TileContext(nc) as tc:
        build_phase_a(tc, outs, ins, cfg)
    nc.compile()
    return nc


def _build_b(dm, pp, group=12):
    nc = _new_nc(dm["M"])
    NW, NB = dm["NW"], pp["NB"]
    ins = dict(
        T=nc.dram_tensor("T", [dm["T_ROWS"], TROW], BF16, kind="ExternalInput").ap(),
        idx16w=nc.dram_tensor("idx16w", [128, NB * 8], I16, kind="ExternalInput").ap(),
        dstloc=nc.dram_tensor("dstloc", [128, NB], F32, kind="ExternalInput").ap(),
        deg_in_sh=nc.dram_tensor("deg_in_sh", [128, NW], F32, kind="ExternalInput").ap(),
        vmat=nc.dram_tensor("vmat", [dm["RANK"], dm["DOUT"]], F32, kind="ExternalInput").ap(),
    )
    outs = dict(out_sh=nc.dram_tensor("out_sh", [NW * 128, dm["DOUT"]], F32,
                                      kind="ExternalOutput").ap())
    cfg = dict(NW=NW, DOUT=dm["DOUT"], RANK=dm["RANK"], BL=pp["BL"], BH=pp["BH"],
               woff=pp["woff"], NB=NB, HI_BASE=dm["HI_BASE"], T_ROWS=dm["T_ROWS"],
               GROUP=group, ni_lo=pp["ni_lo"], ni_hi=pp["ni_hi"])
    with tile.TileContext(nc) as tc:
        build_phase_b(tc, outs, ins, cfg)
    nc.compile()
    return nc


def run_all(feat, w1, w2, v, src, dst, trace=False, tmpdir_a=None, tmpdir_b=None):
    """Returns (output, info dict with per-launch BassKernelResults)."""
    dm = make_dims(N=feat.shape[0], E=src.shape[0], DIN=feat.shape[1],
                   DOUT=w1.shape[1], RANK=v.shape[0])
    pp = preprocess(src, dst, dm)
    M, NSH = dm["M"], dm["NSH"]

    ains = host_phase_a_inputs(feat, w1, w2, pp, dm)
    nc_a = _build_a(dm)
    ra = bass_utils.run_bass_kernel_spmd(nc_a, ains, list(range(M)), trace=trace,
                                         tmpdir=tmpdir_a)
    shards = [ra.results[k]["Tsh"] for k in range(M)]
    T = assemble_T(shards, dm)

    bins = host_phase_b_inputs(T, pp, v, dm)
    nc_b = _build_b(dm, pp)
    rb = bass_utils.run_bass_kernel_spmd(nc_b, bins, list(range(M)), trace=trace,
                                         tmpdir=tmpdir_b)
    out = np.concatenate([rb.results[k]["out_sh"][:NSH] for k in range(M)], axis=0)
    return out.astype(np.float32), dict(ra=ra, rb=rb, dm=dm, pp=pp)


def kernel(feat, w1, w2, v, src, dst):
    feat = np.asarray(feat, np.float32)
    w1 = np.asarray(w1, np.float32)
    w2 = np.asarray(w2, np.float32)
    v = np.asarray(v, np.float32)
    src = np.asarray(src)
    dst = np.asarray(dst)
    out, _ = run_all(feat, w1, w2, v, src, dst, trace=False)
    return out
